# revision 12
# baseline (speedup 1.0000x reference)
"""Linformer-style multihead attention on 8 Trainium2 NeuronCores.

Shapes (hardcoded): B=4, S=8192, D=512, H=8, DK=DV=64, PK=256.

Sharding: core c handles batch b=c//2, sequence half h=c%2 (4096 query rows).
The Linformer K/V projections contract over the FULL sequence, so each core
computes VP = We^T @ value[b] and VF = Wf^T @ value[b] over all 8192 rows
(redundant within a batch-pair, but avoids cross-core collectives).

Key algebra (reassociation): reference computes k = value@Wk then We^T@k.
We instead compute VP = We^T@value (8192-contraction) then kh = VP@Wk
(512-contraction). Biases fold in as rank-1 augmentation rows of the small
matmuls; the output bias bo is applied on the host after the gather.

On-chip pipeline is feature-major: query is transposed during DMA (bf16
x-bar transpose), phase B streams value/We/Wf in growing super-chunks on two
DMA queues (few descriptors, ~620ns trigger cost each) so the PE starts
within a few us and stays fed. The attention stage runs the q projection
per (s-tile, head-pair) interleaved with attention. Score matmuls for a
head pair run row-tiled (dk=64 halves of the PE array, concurrent); the AV
and softmax-denominator matmuls write col-tiled so the pair lands stacked
in one PSUM tile and the softmax normalize (reciprocal + multiply) is one
DVE op per pair instead of per head. Everything PE-side is bf16.
"""

import numpy as np
import ml_dtypes
from contextlib import ExitStack

import concourse.bass as bass
import concourse.bacc as bacc
import concourse.mybir as mybir
import concourse.tile as tile
from concourse import bass_utils
from concourse.masks import make_identity

B, S, D = 4, 8192, 512
H, DK, DV, PK = 8, 64, 64, 256
SH = S // 2  # per-core query rows
NCORES = 8
P = 128

F32 = mybir.dt.float32
BF16 = mybir.dt.bfloat16
AF = mybir.ActivationFunctionType
OP = mybir.AluOpType

_CACHE = {}

# phase-B super-chunk sizes in 512-row n-blocks (sum = 16); first ones small
# so the PE starts early
SCHUNKS = [1, 1, 2, 2, 2, 2, 2, 2, 2]


def _build_kernel():
    nc = bacc.Bacc(
        trn_type="TRN2",
        target_bir_lowering=False,
        debug=False,
        num_devices=NCORES,
    )

    q_t = nc.dram_tensor("q", [SH, D], BF16, kind="ExternalInput").ap()
    v_t = nc.dram_tensor("v", [S, D], BF16, kind="ExternalInput").ap()
    we_t = nc.dram_tensor("we", [S, PK], BF16, kind="ExternalInput").ap()
    wf_t = nc.dram_tensor("wf", [S, PK], BF16, kind="ExternalInput").ap()
    wq_t = nc.dram_tensor("wq", [D, D], BF16, kind="ExternalInput").ap()
    wk_t = nc.dram_tensor("wk", [D, D], BF16, kind="ExternalInput").ap()
    wv_t = nc.dram_tensor("wv", [D, D], BF16, kind="ExternalInput").ap()
    wo_t = nc.dram_tensor("wo", [D, D], BF16, kind="ExternalInput").ap()
    wkaug_t = nc.dram_tensor("wkaug", [2, D], BF16, kind="ExternalInput").ap()
    auge_t = nc.dram_tensor("auge", [2, PK], BF16, kind="ExternalInput").ap()
    wvaug_t = nc.dram_tensor("wvaug", [2, D], BF16, kind="ExternalInput").ap()
    augf_t = nc.dram_tensor("augf", [2, PK], BF16, kind="ExternalInput").ap()
    bq_t = nc.dram_tensor("bq", [D], F32, kind="ExternalInput").ap()
    out_t = nc.dram_tensor("out", [SH, D], F32, kind="ExternalOutput").ap()

    NT = SH // 512  # 8 s-tiles of 512

    with ExitStack() as ctx:
        tc = ctx.enter_context(tile.TileContext(nc))
        consts = ctx.enter_context(tc.tile_pool(name="consts", bufs=1))
        big = ctx.enter_context(tc.tile_pool(name="big", bufs=1))

        # ---- persistent activations ----
        qTraw = big.tile([P, 4, SH], BF16)   # query, feature-major
        khT = big.tile([P, 4, PK], BF16)     # [dk(2 heads/row-block), pair, pk]
        vh_sb = big.tile([P, 2, H, DV], BF16)  # [pk rows, chunk, head, dv]
        vpT = big.tile([P, 4, PK], BF16)
        vfT = big.tile([P, 4, PK], BF16)
        vp_sb = big.tile([P, 2, D], BF16)
        vf_sb = big.tile([P, 2, D], BF16)

        # ---- constants / weights on the sync queue (small, first) ----
        wq_sb = consts.tile([P, 4, D], BF16)
        nc.sync.dma_start(out=wq_sb, in_=wq_t.rearrange("(c p) e -> p c e", p=P))
        wk_sb = consts.tile([P, 4, D], BF16)
        nc.sync.dma_start(out=wk_sb, in_=wk_t.rearrange("(c p) e -> p c e", p=P))
        wv_sb = consts.tile([P, 4, D], BF16)
        nc.sync.dma_start(out=wv_sb, in_=wv_t.rearrange("(c p) e -> p c e", p=P))
        wo_sb = consts.tile([P, 4, D], BF16)
        nc.sync.dma_start(out=wo_sb, in_=wo_t.rearrange("(c p) e -> p c e", p=P))
        wkaug_sb = consts.tile([2, D], BF16)
        nc.sync.dma_start(out=wkaug_sb, in_=wkaug_t)
        auge_sb = consts.tile([2, PK], BF16)
        nc.sync.dma_start(out=auge_sb, in_=auge_t)
        wvaug_sb = consts.tile([2, D], BF16)
        nc.sync.dma_start(out=wvaug_sb, in_=wvaug_t)
        augf_sb = consts.tile([2, PK], BF16)
        nc.sync.dma_start(out=augf_sb, in_=augf_t)
        bq_sb = consts.tile([P, 4], F32)
        nc.sync.dma_start(out=bq_sb, in_=bq_t.rearrange("(c p) -> p c", p=P))
        ident_sb = consts.tile([P, P], BF16)
        make_identity(nc, ident_sb)
        ones64 = consts.tile([P, 64], BF16)
        nc.gpsimd.memset(ones64, 1.0)

        # ---- phase B: VP = We^T @ value, VF = Wf^T @ value (full S) ----
        # r=4 row-blocking: partition p of n-block n holds rows n*512+4p+r,
        # so each DMA piece is 4 contiguous rows (4KB for v, 2KB for We/Wf).
        # The contraction is just regrouped; We/v use the same grouping.
        v_r = v_t.rearrange("(n p r) d -> p n (r d)", p=P, r=4)
        we_r = we_t.rearrange("(n p r) k -> p n (r k)", p=P, r=4)
        wf_r = wf_t.rearrange("(n p r) k -> p n (r k)", p=P, r=4)
        NB = 16  # n-blocks of 512 rows
        with (
            tc.tile_pool(name="vstream", bufs=3) as vstream,
            tc.tile_pool(name="wstream", bufs=3) as wstream,
            tc.tile_pool(name="accp", bufs=4, space="PSUM") as accp,
        ):
            vp_ps = [accp.tile([P, D], F32, tag="acc", name=f"vp_ps{i}")
                     for i in range(2)]
            vf_ps = [accp.tile([P, D], F32, tag="acc", name=f"vf_ps{i}")
                     for i in range(2)]
            base = 0
            for si, sc in enumerate(SCHUNKS):
                csl = slice(base, base + sc)
                val_sb = vstream.tile([P, sc, 4, D], BF16, tag="val",
                                      name=f"val{si}", padded_shape=[P, 2, 4, D])
                nc.gpsimd.dma_start(
                    out=val_sb.rearrange("p n r d -> p n (r d)"),
                    in_=v_r[:, csl, :])
                we_sb = wstream.tile([P, sc, 4, PK], BF16, tag="we",
                                     name=f"we{si}", padded_shape=[P, 2, 4, PK])
                nc.gpsimd.dma_start(
                    out=we_sb.rearrange("p n r k -> p n (r k)"),
                    in_=we_r[:, csl, :])
                wf_sb = wstream.tile([P, sc, 4, PK], BF16, tag="wf",
                                     name=f"wf{si}", padded_shape=[P, 2, 4, PK])
                nc.gpsimd.dma_start(
                    out=wf_sb.rearrange("p n r k -> p n (r k)"),
                    in_=wf_r[:, csl, :])
                for i in range(sc):
                    for r in range(4):
                        k = (base + i) * 4 + r
                        first, last = (k == 0), (k == NB * 4 - 1)
                        for ps in range(2):
                            nc.tensor.matmul(
                                vp_ps[ps],
                                lhsT=we_sb[:, i, r, ps * P:(ps + 1) * P],
                                rhs=val_sb[:, i, r, :], start=first, stop=last)
                            nc.tensor.matmul(
                                vf_ps[ps],
                                lhsT=wf_sb[:, i, r, ps * P:(ps + 1) * P],
                                rhs=val_sb[:, i, r, :], start=first, stop=last)
                base += sc
            for ps in range(2):
                nc.vector.tensor_copy(out=vp_sb[:, ps, :], in_=vp_ps[ps])
                nc.vector.tensor_copy(out=vf_sb[:, ps, :], in_=vf_ps[ps])

        # query transpose during DMA (bf16 x-bar; queued on sync after the
        # phase-B We/Wf streams, done before attention starts)
        for dc in range(4):
            nc.sync.dma_start(
                out=qTraw[:, dc, :],
                in_=q_t[:, dc * P:(dc + 1) * P],
                transpose=True,
            )

        # transpose VP/VF to feature-major via PE (full-tile transpose)
        with tc.tile_pool(name="trp", bufs=4, space="PSUM") as trp:
            for ps in range(2):
                for eb in range(4):
                    tp = trp.tile([P, P], BF16, tag="tr", name=f"tp{ps}{eb}")
                    nc.tensor.transpose(
                        out=tp, in_=vp_sb[:, ps, eb * P:(eb + 1) * P],
                        identity=ident_sb)
                    nc.vector.tensor_copy(
                        out=vpT[:, eb, ps * P:(ps + 1) * P], in_=tp)
                    tf = trp.tile([P, P], BF16, tag="tr", name=f"tf{ps}{eb}")
                    nc.tensor.transpose(
                        out=tf, in_=vf_sb[:, ps, eb * P:(eb + 1) * P],
                        identity=ident_sb)
                    nc.vector.tensor_copy(
                        out=vfT[:, eb, ps * P:(ps + 1) * P], in_=tf)

        # khT[e', pk] = Wk^T @ VPT + rank-1 bias rows
        with tc.tile_pool(name="khp", bufs=2, space="PSUM") as khp:
            for pr in range(4):
                ps_t = khp.tile([P, PK], F32, tag="kh")
                for ec in range(4):
                    nc.tensor.matmul(
                        ps_t, lhsT=wk_sb[:, ec, pr * P:(pr + 1) * P],
                        rhs=vpT[:, ec, :], start=(ec == 0), stop=False)
                nc.tensor.matmul(
                    ps_t, lhsT=wkaug_sb[:, pr * P:(pr + 1) * P],
                    rhs=auge_sb, start=False, stop=True)
                nc.vector.tensor_copy(out=khT[:, pr, :], in_=ps_t)

        # vh[pk, dv] = VFT^T @ Wv + rank-1 bias rows (seq-major in pk)
        with tc.tile_pool(name="vhp", bufs=2, space="PSUM") as vhp:
            for ps in range(2):
                ps_t = vhp.tile([P, D], F32, tag="vh")
                for ec in range(4):
                    nc.tensor.matmul(
                        ps_t, lhsT=vfT[:, ec, ps * P:(ps + 1) * P],
                        rhs=wv_sb[:, ec, :], start=(ec == 0), stop=False)
                nc.tensor.matmul(
                    ps_t, lhsT=augf_sb[:, ps * P:(ps + 1) * P],
                    rhs=wvaug_sb, start=False, stop=True)
                nc.vector.tensor_copy(
                    out=vh_sb[:, ps, :, :],
                    in_=ps_t.rearrange("p (h v) -> p h v", h=H))

        # ---- attention: per s-tile, q-projection interleaved with
        #      pair-packed scores / softmax / AV / output projection ----
        out_r = out_t.rearrange("(t c p) d -> t p c d", c=4, p=P)
        with (
            tc.tile_pool(name="mm1", bufs=2, space="PSUM") as mm1,     # 2 banks
            tc.tile_pool(name="scp", bufs=2, space="PSUM") as scp,     # 2 banks
            tc.tile_pool(name="nzp", bufs=2, space="PSUM") as nzp,     # 4 banks
            tc.tile_pool(name="qstp", bufs=2) as qstp,
            tc.tile_pool(name="epool", bufs=6) as epool,
            tc.tile_pool(name="rzp", bufs=2) as rzp,
            tc.tile_pool(name="avp", bufs=2) as avp,
            tc.tile_pool(name="ostage", bufs=2) as ostage,
        ):
            for st in range(NT):
                ssl = slice(st * 512, (st + 1) * 512)
                qst = qstp.tile([P, 4, 512], BF16, tag="qst")
                av_sb = avp.tile([P, 4, 512], BF16, tag="av")
                # q projection for all four e-blocks first, so the vector
                # queue's qst copies run ahead of this s-tile's recip/mult
                # chain (in-order engine queues would otherwise serialize
                # pair j+1's scores behind pair j's softmax).
                for j in range(4):
                    qt = mm1.tile([P, 512], F32, tag="m1", name=f"qt{st}_{j}")
                    for dc in range(4):
                        nc.tensor.matmul(
                            qt, lhsT=wq_sb[:, dc, j * P:(j + 1) * P],
                            rhs=qTraw[:, dc, ssl],
                            start=(dc == 0), stop=(dc == 3))
                    nc.vector.tensor_scalar(
                        out=qst[:, j, :], in0=qt,
                        scalar1=bq_sb[:, j:j + 1], scalar2=None, op0=OP.add)
                for j in range(4):  # head pair (2j, 2j+1)
                    # scores for the pair: row-tiled (dk halves, concurrent),
                    # split by pk-half (ps) so exp/AV pipeline per chunk
                    es = []
                    for ps in range(2):
                        psl = slice(ps * P, (ps + 1) * P)
                        scA = scp.tile([P, 512], F32, tag="sc",
                                       name=f"scA{st}_{j}_{ps}")
                        scB = scp.tile([P, 512], F32, tag="sc",
                                       name=f"scB{st}_{j}_{ps}")
                        nc.tensor.matmul(
                            scA, lhsT=khT[0:64, j, psl],
                            rhs=qst[0:64, j, :], start=True, stop=True,
                            tile_position=(0, 0))
                        nc.tensor.matmul(
                            scB, lhsT=khT[64:P, j, psl],
                            rhs=qst[64:P, j, :], start=True, stop=True,
                            tile_position=(64, 0))
                        eA = epool.tile([P, 512], BF16, tag="e",
                                        name=f"eA{st}_{j}_{ps}")
                        eB = epool.tile([P, 512], BF16, tag="e",
                                        name=f"eB{st}_{j}_{ps}")
                        nc.scalar.activation(out=eA, in_=scA, func=AF.Exp)
                        nc.scalar.activation(out=eB, in_=scB, func=AF.Exp)
                        es.append((eA, eB))
                    # AV + denominator: pair stacked into one PSUM tile;
                    # nz[:,0,:] = numerator, nz[:,1,:] = Z (dup x64)
                    nz = nzp.tile([P, 2, 512], F32, tag="nz", name=f"nz{st}_{j}")
                    for c in range(2):
                        fl, ll = (c == 0), (c == 1)
                        eA, eB = es[c]
                        nc.tensor.matmul(
                            nz[0:64, 0, :], lhsT=vh_sb[:, c, 2 * j, :],
                            rhs=eA, start=fl, stop=ll, tile_position=(0, 0))
                        nc.tensor.matmul(
                            nz[64:P, 0, :], lhsT=vh_sb[:, c, 2 * j + 1, :],
                            rhs=eB, start=fl, stop=ll, tile_position=(0, 64))
                        nc.tensor.matmul(
                            nz[0:64, 1, :], lhsT=ones64[:, :],
                            rhs=eA, start=fl, stop=ll, tile_position=(0, 0))
                        nc.tensor.matmul(
                            nz[64:P, 1, :], lhsT=ones64[:, :],
                            rhs=eB, start=fl, stop=ll, tile_position=(0, 64))
                    rz = rzp.tile([P, 512], F32, tag="rz", name=f"rz{st}_{j}")
                    nc.vector.reciprocal_approx_fast(out=rz, in_=nz[:, 1, :])
                    nc.vector.tensor_tensor(
                        out=av_sb[:, j, :], in0=nz[:, 0, :], in1=rz,
                        op=OP.mult)
                # output projection for the s-tile (bo added on host)
                o_sb = ostage.tile([P, 4, D], F32, tag="ost")
                for sl in range(4):
                    o_t = mm1.tile([P, D], F32, tag="m1", name=f"ot{st}_{sl}")
                    for pr in range(4):
                        nc.tensor.matmul(
                            o_t, lhsT=av_sb[:, pr, sl * P:(sl + 1) * P],
                            rhs=wo_sb[:, pr, :], start=(pr == 0), stop=(pr == 3))
                    nc.vector.tensor_copy(out=o_sb[:, sl, :], in_=o_t)
                nc.sync.dma_start(out=out_r[st], in_=o_sb)

    nc.finalize()
    return nc


def _prep_inputs(inputs):
    bf = ml_dtypes.bfloat16
    f32 = np.float32
    q = np.ascontiguousarray(inputs["query"])
    v = np.ascontiguousarray(inputs["value"])
    We, Wf = np.asarray(inputs["We"]), np.asarray(inputs["Wf"])
    scale = np.float32(DK ** -0.5)
    ones = np.ones(D, f32)
    sWe = We.astype(f32).sum(0)
    sWf = Wf.astype(f32).sum(0)
    shared = {
        "we": We.astype(bf),
        "wf": Wf.astype(bf),
        "wq": (np.asarray(inputs["Wq"]) * scale).astype(bf),
        "wk": np.asarray(inputs["Wk"]).astype(bf),
        "wv": np.asarray(inputs["Wv"]).astype(bf),
        "wo": np.asarray(inputs["Wo"]).astype(bf),
        "wkaug": np.stack([np.asarray(inputs["bk"], f32), ones]).astype(bf),
        "auge": np.stack([sWe, np.asarray(inputs["be"], f32)]).astype(bf),
        "wvaug": np.stack([np.asarray(inputs["bv"], f32), ones]).astype(bf),
        "augf": np.stack([sWf, np.asarray(inputs["bf"], f32)]).astype(bf),
        "bq": (np.asarray(inputs["bq"]) * scale).astype(f32),
    }
    in_maps = []
    for c in range(NCORES):
        b, half = c // 2, c % 2
        m = dict(shared)
        m["q"] = np.ascontiguousarray(q[b, half * SH:(half + 1) * SH, :]).astype(bf)
        m["v"] = np.ascontiguousarray(v[b]).astype(bf)
        in_maps.append(m)
    return in_maps


def kernel(**inputs):
    if "nc" not in _CACHE:
        _CACHE["nc"] = _build_kernel()
    nc = _CACHE["nc"]
    in_maps = _prep_inputs(inputs)
    res = bass_utils.run_bass_kernel_spmd(nc, in_maps, core_ids=list(range(NCORES)))
    bo = np.asarray(inputs["bo"], np.float32)
    out = np.empty((B, S, D), np.float32)
    for c in range(NCORES):
        b, half = c // 2, c % 2
        out[b, half * SH:(half + 1) * SH, :] = res.results[c]["out"]
    out += bo
    return out


# revision 16
# speedup vs baseline: 1.1459x; 1.1459x over previous
"""Linformer-style multihead attention on 8 Trainium2 NeuronCores.

Shapes (hardcoded): B=4, S=8192, D=512, H=8, DK=DV=64, PK=256.

Sharding: core c handles batch b=c//2, sequence half h=c%2 (4096 query rows).
The Linformer K/V projections contract over the FULL sequence, so each core
computes VP = We^T @ value[b] and VF = Wf^T @ value[b] over all 8192 rows
(redundant within a batch-pair, but avoids cross-core collectives).

Key algebra (reassociation): reference computes k = value@Wk then We^T@k.
We instead compute VP = We^T@value (8192-contraction) then kh = VP@Wk
(512-contraction). Biases fold in as rank-1 augmentation rows of the small
matmuls; the output bias bo is applied on the host after the gather.

On-chip pipeline is feature-major: query is transposed during DMA (bf16
x-bar transpose), phase B streams value/We/Wf in growing super-chunks on two
DMA queues (few descriptors, ~620ns trigger cost each) so the PE starts
within a few us and stays fed. The attention stage runs the q projection
per (s-tile, head-pair) interleaved with attention. Score matmuls for a
head pair run row-tiled (dk=64 halves of the PE array, concurrent); the AV
and softmax-denominator matmuls write col-tiled so the pair lands stacked
in one PSUM tile and the softmax normalize (reciprocal + multiply) is one
DVE op per pair instead of per head. Everything PE-side is bf16.
"""

import numpy as np
import ml_dtypes
from contextlib import ExitStack

import concourse.bass as bass
import concourse.bacc as bacc
import concourse.mybir as mybir
import concourse.tile as tile
from concourse import bass_utils
from concourse.masks import make_identity

B, S, D = 4, 8192, 512
H, DK, DV, PK = 8, 64, 64, 256
SH = S // 2  # per-core query rows
NCORES = 8
P = 128

F32 = mybir.dt.float32
BF16 = mybir.dt.bfloat16
AF = mybir.ActivationFunctionType
OP = mybir.AluOpType

_CACHE = {}

# phase-B super-chunk sizes in 512-row n-blocks (sum = 16); first ones small
# so the PE starts early
SCHUNKS = [1, 1, 2, 2, 2, 2, 2, 2, 2]


def _build_kernel():
    nc = bacc.Bacc(
        trn_type="TRN2",
        target_bir_lowering=False,
        debug=False,
        num_devices=NCORES,
    )

    q_t = nc.dram_tensor("q", [SH, D], BF16, kind="ExternalInput").ap()
    v_t = nc.dram_tensor("v", [S, D], BF16, kind="ExternalInput").ap()
    we_t = nc.dram_tensor("we", [S, PK], BF16, kind="ExternalInput").ap()
    wf_t = nc.dram_tensor("wf", [S, PK], BF16, kind="ExternalInput").ap()
    wq_t = nc.dram_tensor("wq", [D, D], BF16, kind="ExternalInput").ap()
    wk_t = nc.dram_tensor("wk", [D, D], BF16, kind="ExternalInput").ap()
    wv_t = nc.dram_tensor("wv", [D, D], BF16, kind="ExternalInput").ap()
    wo_t = nc.dram_tensor("wo", [D, D], BF16, kind="ExternalInput").ap()
    wkaug_t = nc.dram_tensor("wkaug", [2, D], BF16, kind="ExternalInput").ap()
    auge_t = nc.dram_tensor("auge", [2, PK], BF16, kind="ExternalInput").ap()
    wvaug_t = nc.dram_tensor("wvaug", [2, D], BF16, kind="ExternalInput").ap()
    augf_t = nc.dram_tensor("augf", [2, PK], BF16, kind="ExternalInput").ap()
    bq_t = nc.dram_tensor("bq", [D], F32, kind="ExternalInput").ap()
    out_t = nc.dram_tensor("out", [SH, D], F32, kind="ExternalOutput").ap()

    NT = SH // 512  # 8 s-tiles of 512

    with ExitStack() as ctx:
        tc = ctx.enter_context(tile.TileContext(nc))
        consts = ctx.enter_context(tc.tile_pool(name="consts", bufs=1))
        big = ctx.enter_context(tc.tile_pool(name="big", bufs=1))

        # ---- persistent activations ----
        qTraw = big.tile([P, 4, SH], BF16)   # query, feature-major
        khT = big.tile([P, 4, PK], BF16)     # [dk(2 heads/row-block), pair, pk]
        vh_sb = big.tile([P, 2, H, DV], BF16)  # [pk rows, chunk, head, dv]
        vpT = big.tile([P, 4, PK], BF16)
        vfT = big.tile([P, 4, PK], BF16)
        vp_sb = big.tile([P, 2, D], BF16)
        vf_sb = big.tile([P, 2, D], BF16)

        # ---- constants / weights on the scalar queue (after its wf stream,
        # emitted below — needed only from the phase-B epilogue onwards) ----
        wq_sb = consts.tile([P, 4, D], BF16)
        wk_sb = consts.tile([P, 4, D], BF16)
        wv_sb = consts.tile([P, 4, D], BF16)
        wo_sb = consts.tile([P, 4, D], BF16)
        wkaug_sb = consts.tile([2, D], BF16)
        auge_sb = consts.tile([2, PK], BF16)
        wvaug_sb = consts.tile([2, D], BF16)
        augf_sb = consts.tile([2, PK], BF16)
        bq_sb = consts.tile([P, 4], F32)
        ident_sb = consts.tile([P, P], BF16)
        make_identity(nc, ident_sb)
        ones64 = consts.tile([P, 64], BF16)
        nc.gpsimd.memset(ones64, 1.0)

        # ---- phase B: VP = We^T @ value, VF = Wf^T @ value (full S) ----
        # r=4 row-blocking: partition p of n-block n holds rows n*512+4p+r,
        # so each DMA piece is 4 contiguous rows (4KB for v, 2KB for We/Wf).
        # The contraction is just regrouped; We/v use the same grouping.
        v_r = v_t.rearrange("(n p r) d -> p n (r d)", p=P, r=4)
        we_r = we_t.rearrange("(n p r) k -> p n (r k)", p=P, r=4)
        wf_r = wf_t.rearrange("(n p r) k -> p n (r k)", p=P, r=4)
        NB = 16  # n-blocks of 512 rows
        with (
            tc.tile_pool(name="vstream", bufs=3) as vstream,
            tc.tile_pool(name="wstream", bufs=3) as wstream,
            tc.tile_pool(name="accp", bufs=4, space="PSUM") as accp,
        ):
            vp_ps = [accp.tile([P, D], F32, tag="acc", name=f"vp_ps{i}")
                     for i in range(2)]
            vf_ps = [accp.tile([P, D], F32, tag="acc", name=f"vf_ps{i}")
                     for i in range(2)]
            base = 0
            for si, sc in enumerate(SCHUNKS):
                csl = slice(base, base + sc)
                val_sb = vstream.tile([P, sc, 4, D], BF16, tag="val",
                                      name=f"val{si}", padded_shape=[P, 2, 4, D])
                nc.gpsimd.dma_start(
                    out=val_sb.rearrange("p n r d -> p n (r d)"),
                    in_=v_r[:, csl, :])
                we_sb = wstream.tile([P, sc, 4, PK], BF16, tag="we",
                                     name=f"we{si}", padded_shape=[P, 2, 4, PK])
                nc.sync.dma_start(
                    out=we_sb.rearrange("p n r k -> p n (r k)"),
                    in_=we_r[:, csl, :])
                wf_sb = wstream.tile([P, sc, 4, PK], BF16, tag="wf",
                                     name=f"wf{si}", padded_shape=[P, 2, 4, PK])
                nc.scalar.dma_start(
                    out=wf_sb.rearrange("p n r k -> p n (r k)"),
                    in_=wf_r[:, csl, :])
                for i in range(sc):
                    for r in range(4):
                        k = (base + i) * 4 + r
                        first, last = (k == 0), (k == NB * 4 - 1)
                        for ps in range(2):
                            nc.tensor.matmul(
                                vp_ps[ps],
                                lhsT=we_sb[:, i, r, ps * P:(ps + 1) * P],
                                rhs=val_sb[:, i, r, :], start=first, stop=last)
                            nc.tensor.matmul(
                                vf_ps[ps],
                                lhsT=wf_sb[:, i, r, ps * P:(ps + 1) * P],
                                rhs=val_sb[:, i, r, :], start=first, stop=last)
                base += sc
            for ps in range(2):
                nc.vector.tensor_copy(out=vp_sb[:, ps, :], in_=vp_ps[ps])
                nc.vector.tensor_copy(out=vf_sb[:, ps, :], in_=vf_ps[ps])

        # weights on the scalar queue after its wf stream (needed from the
        # phase-B epilogue onwards), then the query DMA-transposes
        nc.scalar.dma_start(out=wq_sb, in_=wq_t.rearrange("(c p) e -> p c e", p=P))
        nc.scalar.dma_start(out=wk_sb, in_=wk_t.rearrange("(c p) e -> p c e", p=P))
        nc.scalar.dma_start(out=wv_sb, in_=wv_t.rearrange("(c p) e -> p c e", p=P))
        nc.scalar.dma_start(out=wo_sb, in_=wo_t.rearrange("(c p) e -> p c e", p=P))
        nc.scalar.dma_start(out=wkaug_sb, in_=wkaug_t)
        nc.scalar.dma_start(out=auge_sb, in_=auge_t)
        nc.scalar.dma_start(out=wvaug_sb, in_=wvaug_t)
        nc.scalar.dma_start(out=augf_sb, in_=augf_t)
        nc.scalar.dma_start(out=bq_sb, in_=bq_t.rearrange("(c p) -> p c", p=P))
        for dc in range(4):
            nc.scalar.dma_start(
                out=qTraw[:, dc, :],
                in_=q_t[:, dc * P:(dc + 1) * P],
                transpose=True,
            )

        # transpose VP/VF to feature-major via PE (full-tile transpose)
        with tc.tile_pool(name="trp", bufs=4, space="PSUM") as trp:
            for ps in range(2):
                for eb in range(4):
                    tp = trp.tile([P, P], BF16, tag="tr", name=f"tp{ps}{eb}")
                    nc.tensor.transpose(
                        out=tp, in_=vp_sb[:, ps, eb * P:(eb + 1) * P],
                        identity=ident_sb)
                    nc.vector.tensor_copy(
                        out=vpT[:, eb, ps * P:(ps + 1) * P], in_=tp)
                    tf = trp.tile([P, P], BF16, tag="tr", name=f"tf{ps}{eb}")
                    nc.tensor.transpose(
                        out=tf, in_=vf_sb[:, ps, eb * P:(eb + 1) * P],
                        identity=ident_sb)
                    nc.vector.tensor_copy(
                        out=vfT[:, eb, ps * P:(ps + 1) * P], in_=tf)

        # khT[e', pk] = Wk^T @ VPT + rank-1 bias rows
        with tc.tile_pool(name="khp", bufs=2, space="PSUM") as khp:
            for pr in range(4):
                ps_t = khp.tile([P, PK], F32, tag="kh")
                for ec in range(4):
                    nc.tensor.matmul(
                        ps_t, lhsT=wk_sb[:, ec, pr * P:(pr + 1) * P],
                        rhs=vpT[:, ec, :], start=(ec == 0), stop=False)
                nc.tensor.matmul(
                    ps_t, lhsT=wkaug_sb[:, pr * P:(pr + 1) * P],
                    rhs=auge_sb, start=False, stop=True)
                nc.vector.tensor_copy(out=khT[:, pr, :], in_=ps_t)

        # vh[pk, dv] = VFT^T @ Wv + rank-1 bias rows (seq-major in pk)
        with tc.tile_pool(name="vhp", bufs=2, space="PSUM") as vhp:
            for ps in range(2):
                ps_t = vhp.tile([P, D], F32, tag="vh")
                for ec in range(4):
                    nc.tensor.matmul(
                        ps_t, lhsT=vfT[:, ec, ps * P:(ps + 1) * P],
                        rhs=wv_sb[:, ec, :], start=(ec == 0), stop=False)
                nc.tensor.matmul(
                    ps_t, lhsT=augf_sb[:, ps * P:(ps + 1) * P],
                    rhs=wvaug_sb, start=False, stop=True)
                nc.vector.tensor_copy(
                    out=vh_sb[:, ps, :, :],
                    in_=ps_t.rearrange("p (h v) -> p h v", h=H))

        # ---- attention: per s-tile, q-projection interleaved with
        #      pair-packed scores / softmax / AV / output projection.
        # One shared 4-slot PSUM ring carries qh, score, and Wo tiles (all
        # [P,512]); nz gets 2 double-bank slots; total exactly 8 banks.
        # Emission is software-pipelined: scores run up to two pairs ahead
        # of the AV matmuls so the serial exp latency of pair j overlaps
        # PE work of pairs j+1/j+2 despite in-order engine queues.
        out_r = out_t.rearrange("(t c p) d -> t p c d", c=4, p=P)
        with (
            tc.tile_pool(name="spool", bufs=4, space="PSUM") as spool,  # 4 banks
            tc.tile_pool(name="nzp", bufs=2, space="PSUM") as nzp,      # 4 banks
            tc.tile_pool(name="qstp", bufs=2) as qstp,
            tc.tile_pool(name="epool", bufs=8) as epool,
            tc.tile_pool(name="rzp", bufs=2) as rzp,
            tc.tile_pool(name="avp", bufs=2) as avp,
            tc.tile_pool(name="ostage", bufs=2) as ostage,
        ):
            for st in range(NT):
                ssl = slice(st * 512, (st + 1) * 512)
                qst = qstp.tile([P, 4, 512], BF16, tag="qst")
                av_sb = avp.tile([P, 4, 512], BF16, tag="av")

                def emit_qh(j):
                    qt = spool.tile([P, 512], F32, tag="s", name=f"qt{st}_{j}")
                    for dc in range(4):
                        nc.tensor.matmul(
                            qt, lhsT=wq_sb[:, dc, j * P:(j + 1) * P],
                            rhs=qTraw[:, dc, ssl],
                            start=(dc == 0), stop=(dc == 3))
                    nc.vector.tensor_scalar(
                        out=qst[:, j, :], in0=qt,
                        scalar1=bq_sb[:, j:j + 1], scalar2=None, op0=OP.add)

                es = {}

                def emit_scores(j):
                    pe = []
                    for ps in range(2):
                        psl = slice(ps * P, (ps + 1) * P)
                        scA = spool.tile([P, 512], F32, tag="s",
                                         name=f"scA{st}_{j}_{ps}")
                        scB = spool.tile([P, 512], F32, tag="s",
                                         name=f"scB{st}_{j}_{ps}")
                        nc.tensor.matmul(
                            scA, lhsT=khT[0:64, j, psl],
                            rhs=qst[0:64, j, :], start=True, stop=True,
                            tile_position=(0, 0))
                        nc.tensor.matmul(
                            scB, lhsT=khT[64:P, j, psl],
                            rhs=qst[64:P, j, :], start=True, stop=True,
                            tile_position=(64, 0))
                        eA = epool.tile([P, 512], BF16, tag="e",
                                        name=f"eA{st}_{j}_{ps}")
                        eB = epool.tile([P, 512], BF16, tag="e",
                                        name=f"eB{st}_{j}_{ps}")
                        nc.scalar.activation(out=eA, in_=scA, func=AF.Exp)
                        nc.scalar.activation(out=eB, in_=scB, func=AF.Exp)
                        pe.append((eA, eB))
                    es[j] = pe

                def emit_av(j):
                    # AV + denominator: pair stacked into one PSUM tile;
                    # nz[:,0,:] = numerator, nz[:,1,:] = Z (dup x64)
                    nz = nzp.tile([P, 2, 512], F32, tag="nz", name=f"nz{st}_{j}")
                    for c in range(2):
                        fl, ll = (c == 0), (c == 1)
                        eA, eB = es[j][c]
                        nc.tensor.matmul(
                            nz[0:64, 0, :], lhsT=vh_sb[:, c, 2 * j, :],
                            rhs=eA, start=fl, stop=ll, tile_position=(0, 0))
                        nc.tensor.matmul(
                            nz[64:P, 0, :], lhsT=vh_sb[:, c, 2 * j + 1, :],
                            rhs=eB, start=fl, stop=ll, tile_position=(0, 64))
                        nc.tensor.matmul(
                            nz[0:64, 1, :], lhsT=ones64[:, :],
                            rhs=eA, start=fl, stop=ll, tile_position=(0, 0))
                        nc.tensor.matmul(
                            nz[64:P, 1, :], lhsT=ones64[:, :],
                            rhs=eB, start=fl, stop=ll, tile_position=(0, 64))
                    rz = rzp.tile([P, 512], F32, tag="rz", name=f"rz{st}_{j}")
                    nc.vector.reciprocal_approx_fast(out=rz, in_=nz[:, 1, :])
                    nc.vector.tensor_tensor(
                        out=av_sb[:, j, :], in0=nz[:, 0, :], in1=rz,
                        op=OP.mult)

                for j in range(4):
                    emit_qh(j)
                emit_scores(0)
                emit_scores(1)
                emit_scores(2)
                emit_av(0)
                emit_scores(3)
                emit_av(1)
                emit_av(2)
                emit_av(3)
                # output projection for the s-tile (bo added on host)
                o_sb = ostage.tile([P, 4, D], F32, tag="ost")
                for sl in range(4):
                    o_t = spool.tile([P, D], F32, tag="s", name=f"ot{st}_{sl}")
                    for pr in range(4):
                        nc.tensor.matmul(
                            o_t, lhsT=av_sb[:, pr, sl * P:(sl + 1) * P],
                            rhs=wo_sb[:, pr, :], start=(pr == 0), stop=(pr == 3))
                    nc.vector.tensor_copy(out=o_sb[:, sl, :], in_=o_t)
                nc.sync.dma_start(out=out_r[st], in_=o_sb)

    nc.finalize()
    return nc


def _prep_inputs(inputs):
    bf = ml_dtypes.bfloat16
    f32 = np.float32
    q = np.ascontiguousarray(inputs["query"])
    v = np.ascontiguousarray(inputs["value"])
    We, Wf = np.asarray(inputs["We"]), np.asarray(inputs["Wf"])
    scale = np.float32(DK ** -0.5)
    ones = np.ones(D, f32)
    sWe = We.astype(f32).sum(0)
    sWf = Wf.astype(f32).sum(0)
    shared = {
        "we": We.astype(bf),
        "wf": Wf.astype(bf),
        "wq": (np.asarray(inputs["Wq"]) * scale).astype(bf),
        "wk": np.asarray(inputs["Wk"]).astype(bf),
        "wv": np.asarray(inputs["Wv"]).astype(bf),
        "wo": np.asarray(inputs["Wo"]).astype(bf),
        "wkaug": np.stack([np.asarray(inputs["bk"], f32), ones]).astype(bf),
        "auge": np.stack([sWe, np.asarray(inputs["be"], f32)]).astype(bf),
        "wvaug": np.stack([np.asarray(inputs["bv"], f32), ones]).astype(bf),
        "augf": np.stack([sWf, np.asarray(inputs["bf"], f32)]).astype(bf),
        "bq": (np.asarray(inputs["bq"]) * scale).astype(f32),
    }
    in_maps = []
    for c in range(NCORES):
        b, half = c // 2, c % 2
        m = dict(shared)
        m["q"] = np.ascontiguousarray(q[b, half * SH:(half + 1) * SH, :]).astype(bf)
        m["v"] = np.ascontiguousarray(v[b]).astype(bf)
        in_maps.append(m)
    return in_maps


def kernel(**inputs):
    if "nc" not in _CACHE:
        _CACHE["nc"] = _build_kernel()
    nc = _CACHE["nc"]
    in_maps = _prep_inputs(inputs)
    res = bass_utils.run_bass_kernel_spmd(nc, in_maps, core_ids=list(range(NCORES)))
    bo = np.asarray(inputs["bo"], np.float32)
    out = np.empty((B, S, D), np.float32)
    for c in range(NCORES):
        b, half = c // 2, c % 2
        out[b, half * SH:(half + 1) * SH, :] = res.results[c]["out"]
    out += bo
    return out


# revision 18
# speedup vs baseline: 1.3806x; 1.2048x over previous
"""Linformer-style multihead attention on 8 Trainium2 NeuronCores.

Shapes (hardcoded): B=4, S=8192, D=512, H=8, DK=DV=64, PK=256.

Sharding: core c handles batch b=c//2, sequence half h=c%2 (4096 query rows).
The Linformer K/V projections contract over the FULL sequence, so each core
computes VP = We^T @ value[b] and VF = Wf^T @ value[b] over all 8192 rows
(redundant within a batch-pair, but avoids cross-core collectives).

Key algebra (reassociation): reference computes k = value@Wk then We^T@k.
We instead compute VP = We^T@value (8192-contraction) then kh = VP@Wk
(512-contraction). Biases fold in as rank-1 augmentation rows of the small
matmuls; the output bias bo is applied on the host after the gather.

On-chip pipeline is feature-major: query is transposed during DMA (bf16
x-bar transpose), phase B streams value/We/Wf in growing super-chunks on two
DMA queues (few descriptors, ~620ns trigger cost each) so the PE starts
within a few us and stays fed. The attention stage runs the q projection
per (s-tile, head-pair) interleaved with attention. Score matmuls for a
head pair run row-tiled (dk=64 halves of the PE array, concurrent); the AV
and softmax-denominator matmuls write col-tiled so the pair lands stacked
in one PSUM tile and the softmax normalize (reciprocal + multiply) is one
DVE op per pair instead of per head. Everything PE-side is bf16.
"""

import numpy as np
import ml_dtypes
from contextlib import ExitStack

import concourse.bass as bass
import concourse.bacc as bacc
import concourse.mybir as mybir
import concourse.tile as tile
from concourse import bass_utils
from concourse.masks import make_identity

B, S, D = 4, 8192, 512
H, DK, DV, PK = 8, 64, 64, 256
SH = S // 2  # per-core query rows
NCORES = 8
P = 128

F32 = mybir.dt.float32
BF16 = mybir.dt.bfloat16
AF = mybir.ActivationFunctionType
OP = mybir.AluOpType

_CACHE = {}

# phase-B super-chunk sizes in 512-row n-blocks (sum = 16); first ones small
# so the PE starts early
SCHUNKS = [1, 1, 2, 2, 2, 2, 2, 2, 2]


def _build_kernel():
    nc = bacc.Bacc(
        trn_type="TRN2",
        target_bir_lowering=False,
        debug=False,
        num_devices=NCORES,
    )

    q_t = nc.dram_tensor("q", [SH, D], BF16, kind="ExternalInput").ap()
    v_t = nc.dram_tensor("v", [SH, D], BF16, kind="ExternalInput").ap()
    we_t = nc.dram_tensor("we", [SH, PK], BF16, kind="ExternalInput").ap()
    wf_t = nc.dram_tensor("wf", [SH, PK], BF16, kind="ExternalInput").ap()
    wq_t = nc.dram_tensor("wq", [D, D], BF16, kind="ExternalInput").ap()
    wk_t = nc.dram_tensor("wk", [D, D], BF16, kind="ExternalInput").ap()
    wv_t = nc.dram_tensor("wv", [D, D], BF16, kind="ExternalInput").ap()
    wo_t = nc.dram_tensor("wo", [D, D], BF16, kind="ExternalInput").ap()
    wkaug_t = nc.dram_tensor("wkaug", [2, D], BF16, kind="ExternalInput").ap()
    auge_t = nc.dram_tensor("auge", [2, PK], BF16, kind="ExternalInput").ap()
    wvaug_t = nc.dram_tensor("wvaug", [2, D], BF16, kind="ExternalInput").ap()
    augf_t = nc.dram_tensor("augf", [2, PK], BF16, kind="ExternalInput").ap()
    bq_t = nc.dram_tensor("bq", [D], F32, kind="ExternalInput").ap()
    out_t = nc.dram_tensor("out", [SH, D], F32, kind="ExternalOutput").ap()

    NT = SH // 512  # 8 s-tiles of 512

    with ExitStack() as ctx:
        tc = ctx.enter_context(tile.TileContext(nc))
        consts = ctx.enter_context(tc.tile_pool(name="consts", bufs=1))
        big = ctx.enter_context(tc.tile_pool(name="big", bufs=1))

        # ---- persistent activations ----
        qTraw = big.tile([P, 4, SH], BF16)   # query, feature-major
        khT = big.tile([P, 4, PK], BF16)     # [dk(2 heads/row-block), pair, pk]
        vh_sb = big.tile([P, 2, H, DV], BF16)  # [pk rows, chunk, head, dv]
        vpT = big.tile([P, 4, PK], BF16)
        vfT = big.tile([P, 4, PK], BF16)
        vp_sb = big.tile([P, 2, D], BF16)
        vf_sb = big.tile([P, 2, D], BF16)

        # ---- constants / weights on the scalar queue (after its wf stream,
        # emitted below — needed only from the phase-B epilogue onwards) ----
        wq_sb = consts.tile([P, 4, D], BF16)
        wk_sb = consts.tile([P, 4, D], BF16)
        wv_sb = consts.tile([P, 4, D], BF16)
        wo_sb = consts.tile([P, 4, D], BF16)
        wkaug_sb = consts.tile([2, D], BF16)
        auge_sb = consts.tile([2, PK], BF16)
        wvaug_sb = consts.tile([2, D], BF16)
        augf_sb = consts.tile([2, PK], BF16)
        bq_sb = consts.tile([P, 4], F32)
        ident_sb = consts.tile([P, P], BF16)
        make_identity(nc, ident_sb)
        ones64 = consts.tile([P, 64], BF16)
        nc.gpsimd.memset(ones64, 1.0)

        # ---- phase B: VP = We^T @ value, VF = Wf^T @ value over THIS
        # core's half of the sequence; the batch-pair partner handles the
        # other half and the partial khT/vh are pair-AllReduced below.
        # r=4 row-blocking: partition p of n-block n holds rows n*512+4p+r,
        # so each DMA piece is 4 contiguous rows (4KB for v, 2KB for We/Wf).
        # All stream tiles are live simultaneously (no ring), so every DMA
        # issues immediately; v/we/wf ride three different queues.
        v_r = v_t.rearrange("(n p r) d -> p n (r d)", p=P, r=4)
        we_r = we_t.rearrange("(n p r) k -> p n (r k)", p=P, r=4)
        wf_r = wf_t.rearrange("(n p r) k -> p n (r k)", p=P, r=4)
        NB = 8  # n-blocks of 512 rows in this half
        with (
            tc.tile_pool(name="vstream", bufs=4) as vstream,
            tc.tile_pool(name="wstream", bufs=8) as wstream,
            tc.tile_pool(name="accp", bufs=4, space="PSUM") as accp,
        ):
            vp_ps = [accp.tile([P, D], F32, tag="acc", name=f"vp_ps{i}")
                     for i in range(2)]
            vf_ps = [accp.tile([P, D], F32, tag="acc", name=f"vf_ps{i}")
                     for i in range(2)]
            vts, wes, wfs = [], [], []
            for si in range(4):
                csl = slice(si * 2, si * 2 + 2)
                val_sb = vstream.tile([P, 2, 4, D], BF16, tag="val",
                                      name=f"val{si}")
                nc.gpsimd.dma_start(
                    out=val_sb.rearrange("p n r d -> p n (r d)"),
                    in_=v_r[:, csl, :])
                we_sb = wstream.tile([P, 2, 4, PK], BF16, tag="we",
                                     name=f"we{si}")
                nc.sync.dma_start(
                    out=we_sb.rearrange("p n r k -> p n (r k)"),
                    in_=we_r[:, csl, :])
                wf_sb = wstream.tile([P, 2, 4, PK], BF16, tag="wf",
                                     name=f"wf{si}")
                nc.scalar.dma_start(
                    out=wf_sb.rearrange("p n r k -> p n (r k)"),
                    in_=wf_r[:, csl, :])
                vts.append(val_sb); wes.append(we_sb); wfs.append(wf_sb)
            for si in range(4):
                for i in range(2):
                    for r in range(4):
                        k = (si * 2 + i) * 4 + r
                        first, last = (k == 0), (k == NB * 4 - 1)
                        for ps in range(2):
                            nc.tensor.matmul(
                                vp_ps[ps],
                                lhsT=wes[si][:, i, r, ps * P:(ps + 1) * P],
                                rhs=vts[si][:, i, r, :], start=first, stop=last)
                            nc.tensor.matmul(
                                vf_ps[ps],
                                lhsT=wfs[si][:, i, r, ps * P:(ps + 1) * P],
                                rhs=vts[si][:, i, r, :], start=first, stop=last)
            for ps in range(2):
                nc.vector.tensor_copy(out=vp_sb[:, ps, :], in_=vp_ps[ps])
                nc.vector.tensor_copy(out=vf_sb[:, ps, :], in_=vf_ps[ps])

        # weights on the scalar queue after its wf stream (needed from the
        # phase-B epilogue onwards), then the query DMA-transposes
        nc.scalar.dma_start(out=wq_sb, in_=wq_t.rearrange("(c p) e -> p c e", p=P))
        nc.scalar.dma_start(out=wk_sb, in_=wk_t.rearrange("(c p) e -> p c e", p=P))
        nc.scalar.dma_start(out=wv_sb, in_=wv_t.rearrange("(c p) e -> p c e", p=P))
        nc.scalar.dma_start(out=wo_sb, in_=wo_t.rearrange("(c p) e -> p c e", p=P))
        nc.scalar.dma_start(out=wkaug_sb, in_=wkaug_t)
        nc.scalar.dma_start(out=auge_sb, in_=auge_t)
        nc.scalar.dma_start(out=wvaug_sb, in_=wvaug_t)
        nc.scalar.dma_start(out=augf_sb, in_=augf_t)
        nc.scalar.dma_start(out=bq_sb, in_=bq_t.rearrange("(c p) -> p c", p=P))
        for dc in range(4):
            nc.scalar.dma_start(
                out=qTraw[:, dc, :],
                in_=q_t[:, dc * P:(dc + 1) * P],
                transpose=True,
            )

        # transpose VP/VF to feature-major via PE (full-tile transpose)
        with tc.tile_pool(name="trp", bufs=4, space="PSUM") as trp:
            for ps in range(2):
                for eb in range(4):
                    tp = trp.tile([P, P], BF16, tag="tr", name=f"tp{ps}{eb}")
                    nc.tensor.transpose(
                        out=tp, in_=vp_sb[:, ps, eb * P:(eb + 1) * P],
                        identity=ident_sb)
                    nc.vector.tensor_copy(
                        out=vpT[:, eb, ps * P:(ps + 1) * P], in_=tp)
                    tf = trp.tile([P, P], BF16, tag="tr", name=f"tf{ps}{eb}")
                    nc.tensor.transpose(
                        out=tf, in_=vf_sb[:, ps, eb * P:(eb + 1) * P],
                        identity=ident_sb)
                    nc.vector.tensor_copy(
                        out=vfT[:, eb, ps * P:(ps + 1) * P], in_=tf)

        # khT[e', pk] = Wk^T @ VPT + rank-1 bias rows
        with tc.tile_pool(name="khp", bufs=2, space="PSUM") as khp:
            for pr in range(4):
                ps_t = khp.tile([P, PK], F32, tag="kh")
                for ec in range(4):
                    nc.tensor.matmul(
                        ps_t, lhsT=wk_sb[:, ec, pr * P:(pr + 1) * P],
                        rhs=vpT[:, ec, :], start=(ec == 0), stop=False)
                nc.tensor.matmul(
                    ps_t, lhsT=wkaug_sb[:, pr * P:(pr + 1) * P],
                    rhs=auge_sb, start=False, stop=True)
                nc.vector.tensor_copy(out=khT[:, pr, :], in_=ps_t)

        # vh[pk, dv] = VFT^T @ Wv + rank-1 bias rows (seq-major in pk)
        with tc.tile_pool(name="vhp", bufs=2, space="PSUM") as vhp:
            for ps in range(2):
                ps_t = vhp.tile([P, D], F32, tag="vh")
                for ec in range(4):
                    nc.tensor.matmul(
                        ps_t, lhsT=vfT[:, ec, ps * P:(ps + 1) * P],
                        rhs=wv_sb[:, ec, :], start=(ec == 0), stop=False)
                nc.tensor.matmul(
                    ps_t, lhsT=augf_sb[:, ps * P:(ps + 1) * P],
                    rhs=wvaug_sb, start=False, stop=True)
                nc.vector.tensor_copy(
                    out=vh_sb[:, ps, :, :],
                    in_=ps_t.rearrange("p (h v) -> p h v", h=H))

        # ---- pair AllReduce of the half-sequence partials of khT and vh
        # (the rank-1 bias rows were halved on the host so the pair sum
        # applies them exactly once) ----
        with tc.tile_pool(name="dramb", bufs=2, space="DRAM") as dramb:
            cc_in = dramb.tile([P, 2048], BF16, name="cc_in")
            cc_out = dramb.tile([P, 2048], BF16, name="cc_out")
            nc.gpsimd.dma_start(out=cc_in[:, 0:1024],
                                in_=khT.rearrange("p a k -> p (a k)"))
            nc.gpsimd.dma_start(out=cc_in[:, 1024:2048],
                                in_=vh_sb.rearrange("p c h v -> p (c h v)"))
            nc.gpsimd.collective_compute(
                "AllReduce", OP.add,
                replica_groups=[[0, 1], [2, 3], [4, 5], [6, 7]],
                ins=[cc_in.opt()], outs=[cc_out.opt()])
            nc.gpsimd.dma_start(out=khT.rearrange("p a k -> p (a k)"),
                                in_=cc_out[:, 0:1024])
            nc.gpsimd.dma_start(out=vh_sb.rearrange("p c h v -> p (c h v)"),
                                in_=cc_out[:, 1024:2048])

        # ---- attention: per s-tile, q-projection interleaved with
        #      pair-packed scores / softmax / AV / output projection.
        # One shared 4-slot PSUM ring carries qh, score, and Wo tiles (all
        # [P,512]); nz gets 2 double-bank slots; total exactly 8 banks.
        # Emission is software-pipelined: scores run up to two pairs ahead
        # of the AV matmuls so the serial exp latency of pair j overlaps
        # PE work of pairs j+1/j+2 despite in-order engine queues.
        out_r = out_t.rearrange("(t c p) d -> t p c d", c=4, p=P)
        with (
            tc.tile_pool(name="spool", bufs=4, space="PSUM") as spool,  # 4 banks
            tc.tile_pool(name="nzp", bufs=2, space="PSUM") as nzp,      # 4 banks
            tc.tile_pool(name="qstp", bufs=2) as qstp,
            tc.tile_pool(name="epool", bufs=8) as epool,
            tc.tile_pool(name="rzp", bufs=2) as rzp,
            tc.tile_pool(name="avp", bufs=2) as avp,
            tc.tile_pool(name="ostage", bufs=2) as ostage,
        ):
            # Wo for s-tile st-1 is emitted in the middle of s-tile st so
            # the PE's output-projection block lands inside the next tile's
            # exp-saturated stretch (the scalar engine is the attention
            # pacer and must never idle).
            def emit_wo(wst, wav):
                o_sb = ostage.tile([P, 4, D], F32, tag="ost",
                                   name=f"osb{wst}")
                for sl in range(4):
                    o_t = spool.tile([P, D], F32, tag="s", name=f"ot{wst}_{sl}")
                    for pr in range(4):
                        nc.tensor.matmul(
                            o_t, lhsT=wav[:, pr, sl * P:(sl + 1) * P],
                            rhs=wo_sb[:, pr, :], start=(pr == 0), stop=(pr == 3))
                    nc.vector.tensor_copy(out=o_sb[:, sl, :], in_=o_t)
                nc.sync.dma_start(out=out_r[wst], in_=o_sb)

            pend = None
            for st in range(NT):
                ssl = slice(st * 512, (st + 1) * 512)
                qst = qstp.tile([P, 4, 512], BF16, tag="qst")
                av_sb = avp.tile([P, 4, 512], BF16, tag="av")

                def emit_qh(j):
                    qt = spool.tile([P, 512], F32, tag="s", name=f"qt{st}_{j}")
                    for dc in range(4):
                        nc.tensor.matmul(
                            qt, lhsT=wq_sb[:, dc, j * P:(j + 1) * P],
                            rhs=qTraw[:, dc, ssl],
                            start=(dc == 0), stop=(dc == 3))
                    nc.vector.tensor_scalar(
                        out=qst[:, j, :], in0=qt,
                        scalar1=bq_sb[:, j:j + 1], scalar2=None, op0=OP.add)

                es = {}

                def emit_scores(j):
                    pe = []
                    for ps in range(2):
                        psl = slice(ps * P, (ps + 1) * P)
                        scA = spool.tile([P, 512], F32, tag="s",
                                         name=f"scA{st}_{j}_{ps}")
                        scB = spool.tile([P, 512], F32, tag="s",
                                         name=f"scB{st}_{j}_{ps}")
                        nc.tensor.matmul(
                            scA, lhsT=khT[0:64, j, psl],
                            rhs=qst[0:64, j, :], start=True, stop=True,
                            tile_position=(0, 0))
                        nc.tensor.matmul(
                            scB, lhsT=khT[64:P, j, psl],
                            rhs=qst[64:P, j, :], start=True, stop=True,
                            tile_position=(64, 0))
                        eA = epool.tile([P, 512], BF16, tag="e",
                                        name=f"eA{st}_{j}_{ps}")
                        eB = epool.tile([P, 512], BF16, tag="e",
                                        name=f"eB{st}_{j}_{ps}")
                        nc.scalar.activation(out=eA, in_=scA, func=AF.Exp)
                        nc.scalar.activation(out=eB, in_=scB, func=AF.Exp)
                        pe.append((eA, eB))
                    es[j] = pe

                def emit_av(j):
                    # AV + denominator: pair stacked into one PSUM tile;
                    # nz[:,0,:] = numerator, nz[:,1,:] = Z (dup x64)
                    nz = nzp.tile([P, 2, 512], F32, tag="nz", name=f"nz{st}_{j}")
                    for c in range(2):
                        fl, ll = (c == 0), (c == 1)
                        eA, eB = es[j][c]
                        nc.tensor.matmul(
                            nz[0:64, 0, :], lhsT=vh_sb[:, c, 2 * j, :],
                            rhs=eA, start=fl, stop=ll, tile_position=(0, 0))
                        nc.tensor.matmul(
                            nz[64:P, 0, :], lhsT=vh_sb[:, c, 2 * j + 1, :],
                            rhs=eB, start=fl, stop=ll, tile_position=(0, 64))
                        nc.tensor.matmul(
                            nz[0:64, 1, :], lhsT=ones64[:, :],
                            rhs=eA, start=fl, stop=ll, tile_position=(0, 0))
                        nc.tensor.matmul(
                            nz[64:P, 1, :], lhsT=ones64[:, :],
                            rhs=eB, start=fl, stop=ll, tile_position=(0, 64))
                    rz = rzp.tile([P, 512], F32, tag="rz", name=f"rz{st}_{j}")
                    nc.vector.reciprocal_approx_fast(out=rz, in_=nz[:, 1, :])
                    nc.vector.tensor_tensor(
                        out=av_sb[:, j, :], in0=nz[:, 0, :], in1=rz,
                        op=OP.mult)

                for j in range(4):
                    emit_qh(j)
                emit_scores(0)
                emit_scores(1)
                if pend is not None:
                    emit_wo(*pend)
                    pend = None
                emit_scores(2)
                emit_av(0)
                emit_scores(3)
                emit_av(1)
                emit_av(2)
                emit_av(3)
                pend = (st, av_sb)
            emit_wo(*pend)

    nc.finalize()
    return nc


def _prep_inputs(inputs):
    bf = ml_dtypes.bfloat16
    f32 = np.float32
    q = np.ascontiguousarray(inputs["query"])
    v = np.ascontiguousarray(inputs["value"])
    We, Wf = np.asarray(inputs["We"]), np.asarray(inputs["Wf"])
    scale = np.float32(DK ** -0.5)
    ones = np.ones(D, f32)
    sWe = We.astype(f32).sum(0)
    sWf = Wf.astype(f32).sum(0)
    # the rank-1 bias rows are applied on BOTH cores of a batch pair and
    # then pair-AllReduced, so they carry a factor 1/2 here
    shared = {
        "wq": (np.asarray(inputs["Wq"]) * scale).astype(bf),
        "wk": np.asarray(inputs["Wk"]).astype(bf),
        "wv": np.asarray(inputs["Wv"]).astype(bf),
        "wo": np.asarray(inputs["Wo"]).astype(bf),
        "wkaug": np.stack([np.asarray(inputs["bk"], f32), ones]).astype(bf),
        "auge": (0.5 * np.stack([sWe, np.asarray(inputs["be"], f32)])).astype(bf),
        "wvaug": np.stack([np.asarray(inputs["bv"], f32), ones]).astype(bf),
        "augf": (0.5 * np.stack([sWf, np.asarray(inputs["bf"], f32)])).astype(bf),
        "bq": (np.asarray(inputs["bq"]) * scale).astype(f32),
    }
    we_h = [np.ascontiguousarray(We[h * SH:(h + 1) * SH]).astype(bf)
            for h in range(2)]
    wf_h = [np.ascontiguousarray(Wf[h * SH:(h + 1) * SH]).astype(bf)
            for h in range(2)]
    in_maps = []
    for c in range(NCORES):
        b, half = c // 2, c % 2
        m = dict(shared)
        m["q"] = np.ascontiguousarray(q[b, half * SH:(half + 1) * SH, :]).astype(bf)
        m["v"] = np.ascontiguousarray(v[b, half * SH:(half + 1) * SH, :]).astype(bf)
        m["we"] = we_h[half]
        m["wf"] = wf_h[half]
        in_maps.append(m)
    return in_maps


def kernel(**inputs):
    if "nc" not in _CACHE:
        _CACHE["nc"] = _build_kernel()
    nc = _CACHE["nc"]
    in_maps = _prep_inputs(inputs)
    res = bass_utils.run_bass_kernel_spmd(nc, in_maps, core_ids=list(range(NCORES)))
    bo = np.asarray(inputs["bo"], np.float32)
    out = np.empty((B, S, D), np.float32)
    for c in range(NCORES):
        b, half = c // 2, c % 2
        out[b, half * SH:(half + 1) * SH, :] = res.results[c]["out"]
    out += bo
    return out


# revision 21
# speedup vs baseline: 1.4900x; 1.0792x over previous
"""Linformer-style multihead attention on 8 Trainium2 NeuronCores.

Shapes (hardcoded): B=4, S=8192, D=512, H=8, DK=DV=64, PK=256.

Sharding: core c handles batch b=c//2, sequence half h=c%2 (4096 query rows).
The Linformer K/V projections contract over the FULL sequence, so each core
computes VP = We^T @ value[b] and VF = Wf^T @ value[b] over all 8192 rows
(redundant within a batch-pair, but avoids cross-core collectives).

Key algebra (reassociation): reference computes k = value@Wk then We^T@k.
We instead compute VP = We^T@value (8192-contraction) then kh = VP@Wk
(512-contraction). Biases fold in as rank-1 augmentation rows of the small
matmuls; the output bias bo is applied on the host after the gather.

On-chip pipeline is feature-major: query is transposed during DMA (bf16
x-bar transpose), phase B streams value/We/Wf in growing super-chunks on two
DMA queues (few descriptors, ~620ns trigger cost each) so the PE starts
within a few us and stays fed. The attention stage runs the q projection
per (s-tile, head-pair) interleaved with attention. Score matmuls for a
head pair run row-tiled (dk=64 halves of the PE array, concurrent); the AV
and softmax-denominator matmuls write col-tiled so the pair lands stacked
in one PSUM tile and the softmax normalize (reciprocal + multiply) is one
DVE op per pair instead of per head. Everything PE-side is bf16.
"""

import numpy as np
import ml_dtypes
from contextlib import ExitStack

import concourse.bass as bass
import concourse.bacc as bacc
import concourse.mybir as mybir
import concourse.tile as tile
from concourse import bass_utils
from concourse.masks import make_identity

B, S, D = 4, 8192, 512
H, DK, DV, PK = 8, 64, 64, 256
SH = S // 2  # per-core query rows
NCORES = 8
P = 128

F32 = mybir.dt.float32
BF16 = mybir.dt.bfloat16
AF = mybir.ActivationFunctionType
OP = mybir.AluOpType

_CACHE = {}

# phase-B super-chunk sizes in 512-row n-blocks (sum = 16); first ones small
# so the PE starts early
SCHUNKS = [1, 1, 2, 2, 2, 2, 2, 2, 2]


def _build_kernel():
    nc = bacc.Bacc(
        trn_type="TRN2",
        target_bir_lowering=False,
        debug=False,
        num_devices=NCORES,
    )

    q_t = nc.dram_tensor("q", [SH, D], BF16, kind="ExternalInput").ap()
    v_t = nc.dram_tensor("v", [SH, D], BF16, kind="ExternalInput").ap()
    we_t = nc.dram_tensor("we", [SH, PK], BF16, kind="ExternalInput").ap()
    wf_t = nc.dram_tensor("wf", [SH, PK], BF16, kind="ExternalInput").ap()
    wq_t = nc.dram_tensor("wq", [D, D], BF16, kind="ExternalInput").ap()
    wk_t = nc.dram_tensor("wk", [D, D], BF16, kind="ExternalInput").ap()
    wv_t = nc.dram_tensor("wv", [D, D], BF16, kind="ExternalInput").ap()
    wo_t = nc.dram_tensor("wo", [D, D], BF16, kind="ExternalInput").ap()
    wkaug_t = nc.dram_tensor("wkaug", [2, D], BF16, kind="ExternalInput").ap()
    auge_t = nc.dram_tensor("auge", [2, PK], BF16, kind="ExternalInput").ap()
    wvaug_t = nc.dram_tensor("wvaug", [2, D], BF16, kind="ExternalInput").ap()
    augf_t = nc.dram_tensor("augf", [2, PK], BF16, kind="ExternalInput").ap()
    bq_t = nc.dram_tensor("bq", [D], F32, kind="ExternalInput").ap()
    out_t = nc.dram_tensor("out", [SH, D], F32, kind="ExternalOutput").ap()

    NT = SH // 512  # 8 s-tiles of 512

    with ExitStack() as ctx:
        tc = ctx.enter_context(tile.TileContext(nc))
        consts = ctx.enter_context(tc.tile_pool(name="consts", bufs=1))
        big = ctx.enter_context(tc.tile_pool(name="big", bufs=1))

        # ---- persistent activations ----
        qTraw = big.tile([P, 4, SH], BF16)   # query, feature-major
        khT = big.tile([P, 4, PK], BF16)     # [dk(2 heads/row-block), pair, pk]
        vh_sb = big.tile([P, 2, H, DV], BF16)  # [pk rows, chunk, head, dv]
        vpT = big.tile([P, 4, PK], BF16)
        vfT = big.tile([P, 4, PK], BF16)
        vp_sb = big.tile([P, 2, D], BF16)
        vf_sb = big.tile([P, 2, D], BF16)

        # ---- constants / weights on the scalar queue (after its wf stream,
        # emitted below — needed only from the phase-B epilogue onwards) ----
        wq_sb = consts.tile([P, 4, D], BF16)
        wk_sb = consts.tile([P, 4, D], BF16)
        wv_sb = consts.tile([P, 4, D], BF16)
        wo_sb = consts.tile([P, 4, D], BF16)
        wkaug_sb = consts.tile([2, D], BF16)
        auge_sb = consts.tile([2, PK], BF16)
        wvaug_sb = consts.tile([2, D], BF16)
        augf_sb = consts.tile([2, PK], BF16)
        bq_sb = consts.tile([P, 4], F32)
        ident_sb = consts.tile([P, P], BF16)
        make_identity(nc, ident_sb)
        ones64 = consts.tile([P, 64], BF16)
        nc.gpsimd.memset(ones64, 1.0)

        # ---- phase B: VP = We^T @ value, VF = Wf^T @ value over THIS
        # core's half of the sequence; the batch-pair partner handles the
        # other half and the partial khT/vh are pair-AllReduced below.
        # r=4 row-blocking: partition p of n-block n holds rows n*512+4p+r,
        # so each DMA piece is 4 contiguous rows (4KB for v, 2KB for We/Wf).
        # All stream tiles are live simultaneously (no ring), so every DMA
        # issues immediately; v/we/wf ride three different queues.
        v_r = v_t.rearrange("(n p r) d -> p n (r d)", p=P, r=4)
        we_r = we_t.rearrange("(n p r) k -> p n (r k)", p=P, r=4)
        wf_r = wf_t.rearrange("(n p r) k -> p n (r k)", p=P, r=4)
        NB = 8  # n-blocks of 512 rows in this half
        with (
            tc.tile_pool(name="vstream", bufs=8) as vstream,
            tc.tile_pool(name="wstream", bufs=16) as wstream,
            tc.tile_pool(name="accp", bufs=4, space="PSUM") as accp,
        ):
            vp_ps = [accp.tile([P, D], F32, tag="acc", name=f"vp_ps{i}")
                     for i in range(2)]
            vf_ps = [accp.tile([P, D], F32, tag="acc", name=f"vf_ps{i}")
                     for i in range(2)]
            vts, wes, wfs = [], [], []
            for si in range(8):
                csl = slice(si, si + 1)
                val_sb = vstream.tile([P, 1, 4, D], BF16, tag="val",
                                      name=f"val{si}")
                nc.gpsimd.dma_start(
                    out=val_sb.rearrange("p n r d -> p n (r d)"),
                    in_=v_r[:, csl, :])
                we_sb = wstream.tile([P, 1, 4, PK], BF16, tag="we",
                                     name=f"we{si}")
                nc.sync.dma_start(
                    out=we_sb.rearrange("p n r k -> p n (r k)"),
                    in_=we_r[:, csl, :])
                wf_sb = wstream.tile([P, 1, 4, PK], BF16, tag="wf",
                                     name=f"wf{si}")
                nc.scalar.dma_start(
                    out=wf_sb.rearrange("p n r k -> p n (r k)"),
                    in_=wf_r[:, csl, :])
                vts.append(val_sb); wes.append(we_sb); wfs.append(wf_sb)
            for si in range(8):
                for r in range(4):
                    k = si * 4 + r
                    first, last = (k == 0), (k == NB * 4 - 1)
                    for ps in range(2):
                        nc.tensor.matmul(
                            vp_ps[ps],
                            lhsT=wes[si][:, 0, r, ps * P:(ps + 1) * P],
                            rhs=vts[si][:, 0, r, :], start=first, stop=last)
                        nc.tensor.matmul(
                            vf_ps[ps],
                            lhsT=wfs[si][:, 0, r, ps * P:(ps + 1) * P],
                            rhs=vts[si][:, 0, r, :], start=first, stop=last)
            for ps in range(2):
                nc.vector.tensor_copy(out=vp_sb[:, ps, :], in_=vp_ps[ps])
                nc.vector.tensor_copy(out=vf_sb[:, ps, :], in_=vf_ps[ps])

        # weights on the scalar queue after its wf stream (needed from the
        # phase-B epilogue onwards), then the query DMA-transposes
        nc.scalar.dma_start(out=wq_sb, in_=wq_t.rearrange("(c p) e -> p c e", p=P))
        nc.scalar.dma_start(out=wk_sb, in_=wk_t.rearrange("(c p) e -> p c e", p=P))
        nc.scalar.dma_start(out=wv_sb, in_=wv_t.rearrange("(c p) e -> p c e", p=P))
        nc.scalar.dma_start(out=wo_sb, in_=wo_t.rearrange("(c p) e -> p c e", p=P))
        nc.scalar.dma_start(out=wkaug_sb, in_=wkaug_t)
        nc.scalar.dma_start(out=auge_sb, in_=auge_t)
        nc.scalar.dma_start(out=wvaug_sb, in_=wvaug_t)
        nc.scalar.dma_start(out=augf_sb, in_=augf_t)
        nc.scalar.dma_start(out=bq_sb, in_=bq_t.rearrange("(c p) -> p c", p=P))
        for dc in range(4):
            nc.sync.dma_start(
                out=qTraw[:, dc, :],
                in_=q_t[:, dc * P:(dc + 1) * P],
                transpose=True,
            )

        # transpose VP/VF to feature-major via PE (full-tile transpose)
        with tc.tile_pool(name="trp", bufs=4, space="PSUM") as trp:
            for ps in range(2):
                for eb in range(4):
                    tp = trp.tile([P, P], BF16, tag="tr", name=f"tp{ps}{eb}")
                    nc.tensor.transpose(
                        out=tp, in_=vp_sb[:, ps, eb * P:(eb + 1) * P],
                        identity=ident_sb)
                    nc.vector.tensor_copy(
                        out=vpT[:, eb, ps * P:(ps + 1) * P], in_=tp)
                    tf = trp.tile([P, P], BF16, tag="tr", name=f"tf{ps}{eb}")
                    nc.tensor.transpose(
                        out=tf, in_=vf_sb[:, ps, eb * P:(eb + 1) * P],
                        identity=ident_sb)
                    nc.vector.tensor_copy(
                        out=vfT[:, eb, ps * P:(ps + 1) * P], in_=tf)

        # khT[e', pk] = Wk^T @ VPT + rank-1 bias rows
        with tc.tile_pool(name="khp", bufs=2, space="PSUM") as khp:
            for pr in range(4):
                ps_t = khp.tile([P, PK], F32, tag="kh")
                for ec in range(4):
                    nc.tensor.matmul(
                        ps_t, lhsT=wk_sb[:, ec, pr * P:(pr + 1) * P],
                        rhs=vpT[:, ec, :], start=(ec == 0), stop=False)
                nc.tensor.matmul(
                    ps_t, lhsT=wkaug_sb[:, pr * P:(pr + 1) * P],
                    rhs=auge_sb, start=False, stop=True)
                nc.vector.tensor_copy(out=khT[:, pr, :], in_=ps_t)

        # vh[pk, dv] = VFT^T @ Wv + rank-1 bias rows (seq-major in pk)
        with tc.tile_pool(name="vhp", bufs=2, space="PSUM") as vhp:
            for ps in range(2):
                ps_t = vhp.tile([P, D], F32, tag="vh")
                for ec in range(4):
                    nc.tensor.matmul(
                        ps_t, lhsT=vfT[:, ec, ps * P:(ps + 1) * P],
                        rhs=wv_sb[:, ec, :], start=(ec == 0), stop=False)
                nc.tensor.matmul(
                    ps_t, lhsT=augf_sb[:, ps * P:(ps + 1) * P],
                    rhs=wvaug_sb, start=False, stop=True)
                nc.vector.tensor_copy(
                    out=vh_sb[:, ps, :, :],
                    in_=ps_t.rearrange("p (h v) -> p h v", h=H))

        # ---- pair AllReduce of the half-sequence partials of khT and vh
        # (the rank-1 bias rows were halved on the host so the pair sum
        # applies them exactly once) ----
        with tc.tile_pool(name="dramb", bufs=2, space="DRAM") as dramb:
            cc_in = dramb.tile([P, 2048], BF16, name="cc_in")
            cc_out = dramb.tile([P, 2048], BF16, name="cc_out")
            nc.gpsimd.dma_start(out=cc_in[:, 0:1024],
                                in_=khT.rearrange("p a k -> p (a k)"))
            nc.gpsimd.dma_start(out=cc_in[:, 1024:2048],
                                in_=vh_sb.rearrange("p c h v -> p (c h v)"))
            nc.gpsimd.collective_compute(
                "AllReduce", OP.add,
                replica_groups=[[0, 1], [2, 3], [4, 5], [6, 7]],
                ins=[cc_in.opt()], outs=[cc_out.opt()])
            nc.gpsimd.dma_start(out=khT.rearrange("p a k -> p (a k)"),
                                in_=cc_out[:, 0:1024])
            nc.gpsimd.dma_start(out=vh_sb.rearrange("p c h v -> p (c h v)"),
                                in_=cc_out[:, 1024:2048])

        # ---- attention: per s-tile, q-projection interleaved with
        #      pair-packed scores / softmax / AV / output projection.
        # One shared 4-slot PSUM ring carries qh, score, and Wo tiles (all
        # [P,512]); nz gets 2 double-bank slots; total exactly 8 banks.
        # Emission is software-pipelined: scores run up to two pairs ahead
        # of the AV matmuls so the serial exp latency of pair j overlaps
        # PE work of pairs j+1/j+2 despite in-order engine queues.
        out_r = out_t.rearrange("(t c p) d -> t p c d", c=4, p=P)
        with (
            tc.tile_pool(name="spool", bufs=4, space="PSUM") as spool,  # 4 banks
            tc.tile_pool(name="nzp", bufs=2, space="PSUM") as nzp,      # 4 banks
            tc.tile_pool(name="qstp", bufs=8) as qstp,
            tc.tile_pool(name="epool", bufs=8) as epool,
            tc.tile_pool(name="rzp", bufs=2) as rzp,
            tc.tile_pool(name="avp", bufs=2) as avp,
            tc.tile_pool(name="ostage", bufs=2) as ostage,
        ):
            # Wo for s-tile st-1 is emitted in the middle of s-tile st so
            # the PE's output-projection block lands inside the next tile's
            # exp-saturated stretch (the scalar engine is the attention
            # pacer and must never idle).
            def emit_wo(wst, wav):
                o_sb = ostage.tile([P, 4, D], F32, tag="ost",
                                   name=f"osb{wst}")
                for sl in range(4):
                    o_t = spool.tile([P, D], F32, tag="s", name=f"ot{wst}_{sl}")
                    for pr in range(4):
                        nc.tensor.matmul(
                            o_t, lhsT=wav[:, pr, sl * P:(sl + 1) * P],
                            rhs=wo_sb[:, pr, :], start=(pr == 0), stop=(pr == 3))
                    nc.vector.tensor_copy(out=o_sb[:, sl, :], in_=o_t)
                nc.sync.dma_start(out=out_r[wst], in_=o_sb)

            # All q-projection chains are hoisted ahead of the s-tile
            # loop: their matmuls fill the PE while the pair-AllReduce of
            # khT/vh is in flight, and their PSUM->SBUF copies (with the
            # bq bias) ride the scalar engine, which is idle until the
            # first exp.
            qsts = []
            for st in range(NT):
                ssl = slice(st * 512, (st + 1) * 512)
                qst = qstp.tile([P, 4, 512], BF16, tag="qst", name=f"qst{st}")
                qsts.append(qst)
                for j in range(4):
                    qt = spool.tile([P, 512], F32, tag="s", name=f"qt{st}_{j}")
                    for dc in range(4):
                        nc.tensor.matmul(
                            qt, lhsT=wq_sb[:, dc, j * P:(j + 1) * P],
                            rhs=qTraw[:, dc, ssl],
                            start=(dc == 0), stop=(dc == 3))
                    nc.scalar.activation(
                        out=qst[:, j, :], in_=qt, func=AF.Identity,
                        bias=bq_sb[:, j:j + 1])

            pend = None
            for st in range(NT):
                qst = qsts[st]
                av_sb = avp.tile([P, 4, 512], BF16, tag="av")

                es = {}

                def emit_scores(j):
                    pe = []
                    for ps in range(2):
                        psl = slice(ps * P, (ps + 1) * P)
                        scA = spool.tile([P, 512], F32, tag="s",
                                         name=f"scA{st}_{j}_{ps}")
                        scB = spool.tile([P, 512], F32, tag="s",
                                         name=f"scB{st}_{j}_{ps}")
                        nc.tensor.matmul(
                            scA, lhsT=khT[0:64, j, psl],
                            rhs=qst[0:64, j, :], start=True, stop=True,
                            tile_position=(0, 0))
                        nc.tensor.matmul(
                            scB, lhsT=khT[64:P, j, psl],
                            rhs=qst[64:P, j, :], start=True, stop=True,
                            tile_position=(64, 0))
                        eA = epool.tile([P, 512], BF16, tag="e",
                                        name=f"eA{st}_{j}_{ps}")
                        eB = epool.tile([P, 512], BF16, tag="e",
                                        name=f"eB{st}_{j}_{ps}")
                        nc.scalar.activation(out=eA, in_=scA, func=AF.Exp)
                        nc.scalar.activation(out=eB, in_=scB, func=AF.Exp)
                        pe.append((eA, eB))
                    es[j] = pe

                def emit_av(j):
                    # AV + denominator: pair stacked into one PSUM tile;
                    # nz[:,0,:] = numerator, nz[:,1,:] = Z (dup x64)
                    nz = nzp.tile([P, 2, 512], F32, tag="nz", name=f"nz{st}_{j}")
                    for c in range(2):
                        fl, ll = (c == 0), (c == 1)
                        eA, eB = es[j][c]
                        nc.tensor.matmul(
                            nz[0:64, 0, :], lhsT=vh_sb[:, c, 2 * j, :],
                            rhs=eA, start=fl, stop=ll, tile_position=(0, 0))
                        nc.tensor.matmul(
                            nz[64:P, 0, :], lhsT=vh_sb[:, c, 2 * j + 1, :],
                            rhs=eB, start=fl, stop=ll, tile_position=(0, 64))
                        nc.tensor.matmul(
                            nz[0:64, 1, :], lhsT=ones64[:, :],
                            rhs=eA, start=fl, stop=ll, tile_position=(0, 0))
                        nc.tensor.matmul(
                            nz[64:P, 1, :], lhsT=ones64[:, :],
                            rhs=eB, start=fl, stop=ll, tile_position=(0, 64))
                    rz = rzp.tile([P, 512], F32, tag="rz", name=f"rz{st}_{j}")
                    nc.vector.reciprocal_approx_fast(out=rz, in_=nz[:, 1, :])
                    nc.vector.tensor_tensor(
                        out=av_sb[:, j, :], in0=nz[:, 0, :], in1=rz,
                        op=OP.mult)

                emit_scores(0)
                emit_scores(1)
                if pend is not None:
                    emit_wo(*pend)
                    pend = None
                emit_scores(2)
                emit_av(0)
                emit_scores(3)
                emit_av(1)
                emit_av(2)
                emit_av(3)
                pend = (st, av_sb)
            emit_wo(*pend)

    nc.finalize()
    return nc


def _prep_inputs(inputs):
    bf = ml_dtypes.bfloat16
    f32 = np.float32
    q = np.ascontiguousarray(inputs["query"])
    v = np.ascontiguousarray(inputs["value"])
    We, Wf = np.asarray(inputs["We"]), np.asarray(inputs["Wf"])
    scale = np.float32(DK ** -0.5)
    ones = np.ones(D, f32)
    sWe = We.astype(f32).sum(0)
    sWf = Wf.astype(f32).sum(0)
    # the rank-1 bias rows are applied on BOTH cores of a batch pair and
    # then pair-AllReduced, so they carry a factor 1/2 here
    shared = {
        "wq": (np.asarray(inputs["Wq"]) * scale).astype(bf),
        "wk": np.asarray(inputs["Wk"]).astype(bf),
        "wv": np.asarray(inputs["Wv"]).astype(bf),
        "wo": np.asarray(inputs["Wo"]).astype(bf),
        "wkaug": np.stack([np.asarray(inputs["bk"], f32), ones]).astype(bf),
        "auge": (0.5 * np.stack([sWe, np.asarray(inputs["be"], f32)])).astype(bf),
        "wvaug": np.stack([np.asarray(inputs["bv"], f32), ones]).astype(bf),
        "augf": (0.5 * np.stack([sWf, np.asarray(inputs["bf"], f32)])).astype(bf),
        "bq": (np.asarray(inputs["bq"]) * scale).astype(f32),
    }
    we_h = [np.ascontiguousarray(We[h * SH:(h + 1) * SH]).astype(bf)
            for h in range(2)]
    wf_h = [np.ascontiguousarray(Wf[h * SH:(h + 1) * SH]).astype(bf)
            for h in range(2)]
    in_maps = []
    for c in range(NCORES):
        b, half = c // 2, c % 2
        m = dict(shared)
        m["q"] = np.ascontiguousarray(q[b, half * SH:(half + 1) * SH, :]).astype(bf)
        m["v"] = np.ascontiguousarray(v[b, half * SH:(half + 1) * SH, :]).astype(bf)
        m["we"] = we_h[half]
        m["wf"] = wf_h[half]
        in_maps.append(m)
    return in_maps


def kernel(**inputs):
    if "nc" not in _CACHE:
        _CACHE["nc"] = _build_kernel()
    nc = _CACHE["nc"]
    in_maps = _prep_inputs(inputs)
    res = bass_utils.run_bass_kernel_spmd(nc, in_maps, core_ids=list(range(NCORES)))
    bo = np.asarray(inputs["bo"], np.float32)
    out = np.empty((B, S, D), np.float32)
    for c in range(NCORES):
        b, half = c // 2, c % 2
        out[b, half * SH:(half + 1) * SH, :] = res.results[c]["out"]
    out += bo
    return out


# revision 22
# speedup vs baseline: 1.5348x; 1.0301x over previous
"""Linformer-style multihead attention on 8 Trainium2 NeuronCores.

Shapes (hardcoded): B=4, S=8192, D=512, H=8, DK=DV=64, PK=256.

Sharding: core c handles batch b=c//2, sequence half h=c%2 (4096 query rows).
The Linformer K/V projections contract over the FULL sequence, so each core
computes VP = We^T @ value[b] and VF = Wf^T @ value[b] over all 8192 rows
(redundant within a batch-pair, but avoids cross-core collectives).

Key algebra (reassociation): reference computes k = value@Wk then We^T@k.
We instead compute VP = We^T@value (8192-contraction) then kh = VP@Wk
(512-contraction). Biases fold in as rank-1 augmentation rows of the small
matmuls; the output bias bo is applied on the host after the gather.

On-chip pipeline is feature-major: query is transposed during DMA (bf16
x-bar transpose), phase B streams value/We/Wf in growing super-chunks on two
DMA queues (few descriptors, ~620ns trigger cost each) so the PE starts
within a few us and stays fed. The attention stage runs the q projection
per (s-tile, head-pair) interleaved with attention. Score matmuls for a
head pair run row-tiled (dk=64 halves of the PE array, concurrent); the AV
and softmax-denominator matmuls write col-tiled so the pair lands stacked
in one PSUM tile and the softmax normalize (reciprocal + multiply) is one
DVE op per pair instead of per head. Everything PE-side is bf16.
"""

import numpy as np
import ml_dtypes
from contextlib import ExitStack

import concourse.bass as bass
import concourse.bacc as bacc
import concourse.mybir as mybir
import concourse.tile as tile
from concourse import bass_utils
from concourse.masks import make_identity

B, S, D = 4, 8192, 512
H, DK, DV, PK = 8, 64, 64, 256
SH = S // 2  # per-core query rows
NCORES = 8
P = 128

F32 = mybir.dt.float32
BF16 = mybir.dt.bfloat16
AF = mybir.ActivationFunctionType
OP = mybir.AluOpType

_CACHE = {}

# phase-B super-chunk sizes in 512-row n-blocks (sum = 16); first ones small
# so the PE starts early
SCHUNKS = [1, 1, 2, 2, 2, 2, 2, 2, 2]


def _build_kernel():
    nc = bacc.Bacc(
        trn_type="TRN2",
        target_bir_lowering=False,
        debug=False,
        num_devices=NCORES,
    )

    q_t = nc.dram_tensor("q", [SH, D], BF16, kind="ExternalInput").ap()
    v_t = nc.dram_tensor("v", [SH, D], BF16, kind="ExternalInput").ap()
    we_t = nc.dram_tensor("we", [SH, PK], BF16, kind="ExternalInput").ap()
    wf_t = nc.dram_tensor("wf", [SH, PK], BF16, kind="ExternalInput").ap()
    wq_t = nc.dram_tensor("wq", [D, D], BF16, kind="ExternalInput").ap()
    wk_t = nc.dram_tensor("wk", [D, D], BF16, kind="ExternalInput").ap()
    wv_t = nc.dram_tensor("wv", [D, D], BF16, kind="ExternalInput").ap()
    wo_t = nc.dram_tensor("wo", [D, D], BF16, kind="ExternalInput").ap()
    wkaug_t = nc.dram_tensor("wkaug", [2, D], BF16, kind="ExternalInput").ap()
    auge_t = nc.dram_tensor("auge", [2, PK], BF16, kind="ExternalInput").ap()
    wvaug_t = nc.dram_tensor("wvaug", [2, D], BF16, kind="ExternalInput").ap()
    augf_t = nc.dram_tensor("augf", [2, PK], BF16, kind="ExternalInput").ap()
    bq_t = nc.dram_tensor("bq", [D], F32, kind="ExternalInput").ap()
    out_t = nc.dram_tensor("out", [SH, D], F32, kind="ExternalOutput").ap()

    NT = SH // 512  # 8 s-tiles of 512

    with ExitStack() as ctx:
        tc = ctx.enter_context(tile.TileContext(nc))
        consts = ctx.enter_context(tc.tile_pool(name="consts", bufs=1))
        big = ctx.enter_context(tc.tile_pool(name="big", bufs=1))

        # ---- persistent activations ----
        qTraw = big.tile([P, 4, SH], BF16)   # query, feature-major
        khT = big.tile([P, 4, PK], BF16)     # [dk(2 heads/row-block), pair, pk]
        vh_sb = big.tile([P, 2, H, DV], BF16)  # [pk rows, chunk, head, dv]
        vpT = big.tile([P, 4, PK], BF16)
        vfT = big.tile([P, 4, PK], BF16)
        vp_sb = big.tile([P, 2, D], BF16)
        vf_sb = big.tile([P, 2, D], BF16)

        # ---- constants / weights on the scalar queue (after its wf stream,
        # emitted below — needed only from the phase-B epilogue onwards) ----
        wq_sb = consts.tile([P, 4, D], BF16)
        wk_sb = consts.tile([P, 4, D], BF16)
        wv_sb = consts.tile([P, 4, D], BF16)
        wo_sb = consts.tile([P, 4, D], BF16)
        wkaug_sb = consts.tile([2, D], BF16)
        auge_sb = consts.tile([2, PK], BF16)
        wvaug_sb = consts.tile([2, D], BF16)
        augf_sb = consts.tile([2, PK], BF16)
        bq_sb = consts.tile([P, 4], F32)
        ident_sb = consts.tile([P, P], BF16)
        make_identity(nc, ident_sb)
        ones64 = consts.tile([P, 64], BF16)
        nc.gpsimd.memset(ones64, 1.0)

        # ---- phase B: VP = We^T @ value, VF = Wf^T @ value over THIS
        # core's half of the sequence; the batch-pair partner handles the
        # other half and the partial khT/vh are pair-AllReduced below.
        # r=4 row-blocking: partition p of n-block n holds rows n*512+4p+r,
        # so each DMA piece is 4 contiguous rows (4KB for v, 2KB for We/Wf).
        # All stream tiles are live simultaneously (no ring), so every DMA
        # issues immediately; v/we/wf ride three different queues.
        v_r = v_t.rearrange("(n p r) d -> p n (r d)", p=P, r=4)
        we_r = we_t.rearrange("(n p r) k -> p n (r k)", p=P, r=4)
        wf_r = wf_t.rearrange("(n p r) k -> p n (r k)", p=P, r=4)
        NB = 8  # n-blocks of 512 rows in this half
        with (
            tc.tile_pool(name="vstream", bufs=8) as vstream,
            tc.tile_pool(name="wstream", bufs=16) as wstream,
            tc.tile_pool(name="accp", bufs=4, space="PSUM") as accp,
        ):
            vp_ps = [accp.tile([P, D], F32, tag="acc", name=f"vp_ps{i}")
                     for i in range(2)]
            vf_ps = [accp.tile([P, D], F32, tag="acc", name=f"vf_ps{i}")
                     for i in range(2)]
            vts, wes, wfs = [], [], []
            for si in range(8):
                csl = slice(si, si + 1)
                val_sb = vstream.tile([P, 1, 4, D], BF16, tag="val",
                                      name=f"val{si}")
                nc.gpsimd.dma_start(
                    out=val_sb.rearrange("p n r d -> p n (r d)"),
                    in_=v_r[:, csl, :])
                we_sb = wstream.tile([P, 1, 4, PK], BF16, tag="we",
                                     name=f"we{si}")
                nc.sync.dma_start(
                    out=we_sb.rearrange("p n r k -> p n (r k)"),
                    in_=we_r[:, csl, :])
                wf_sb = wstream.tile([P, 1, 4, PK], BF16, tag="wf",
                                     name=f"wf{si}")
                nc.scalar.dma_start(
                    out=wf_sb.rearrange("p n r k -> p n (r k)"),
                    in_=wf_r[:, csl, :])
                vts.append(val_sb); wes.append(we_sb); wfs.append(wf_sb)
            for si in range(8):
                for r in range(4):
                    k = si * 4 + r
                    first, last = (k == 0), (k == NB * 4 - 1)
                    for ps in range(2):
                        nc.tensor.matmul(
                            vp_ps[ps],
                            lhsT=wes[si][:, 0, r, ps * P:(ps + 1) * P],
                            rhs=vts[si][:, 0, r, :], start=first, stop=last)
                        nc.tensor.matmul(
                            vf_ps[ps],
                            lhsT=wfs[si][:, 0, r, ps * P:(ps + 1) * P],
                            rhs=vts[si][:, 0, r, :], start=first, stop=last)
            for ps in range(2):
                nc.vector.tensor_copy(out=vp_sb[:, ps, :], in_=vp_ps[ps])
                nc.vector.tensor_copy(out=vf_sb[:, ps, :], in_=vf_ps[ps])

        # weights on the scalar queue after its wf stream (needed from the
        # phase-B epilogue onwards), then the query DMA-transposes
        nc.scalar.dma_start(out=wq_sb, in_=wq_t.rearrange("(c p) e -> p c e", p=P))
        nc.scalar.dma_start(out=wk_sb, in_=wk_t.rearrange("(c p) e -> p c e", p=P))
        nc.scalar.dma_start(out=wv_sb, in_=wv_t.rearrange("(c p) e -> p c e", p=P))
        nc.scalar.dma_start(out=wo_sb, in_=wo_t.rearrange("(c p) e -> p c e", p=P))
        nc.scalar.dma_start(out=wkaug_sb, in_=wkaug_t)
        nc.scalar.dma_start(out=auge_sb, in_=auge_t)
        nc.scalar.dma_start(out=wvaug_sb, in_=wvaug_t)
        nc.scalar.dma_start(out=augf_sb, in_=augf_t)
        nc.scalar.dma_start(out=bq_sb, in_=bq_t.rearrange("(c p) -> p c", p=P))
        for dc in range(4):
            nc.scalar.dma_start(
                out=qTraw[:, dc, :],
                in_=q_t[:, dc * P:(dc + 1) * P],
                transpose=True,
            )

        # transpose VP/VF to feature-major via PE (full-tile transpose)
        with tc.tile_pool(name="trp", bufs=4, space="PSUM") as trp:
            for ps in range(2):
                for eb in range(4):
                    tp = trp.tile([P, P], BF16, tag="tr", name=f"tp{ps}{eb}")
                    nc.tensor.transpose(
                        out=tp, in_=vp_sb[:, ps, eb * P:(eb + 1) * P],
                        identity=ident_sb)
                    nc.vector.tensor_copy(
                        out=vpT[:, eb, ps * P:(ps + 1) * P], in_=tp)
                    tf = trp.tile([P, P], BF16, tag="tr", name=f"tf{ps}{eb}")
                    nc.tensor.transpose(
                        out=tf, in_=vf_sb[:, ps, eb * P:(eb + 1) * P],
                        identity=ident_sb)
                    nc.vector.tensor_copy(
                        out=vfT[:, eb, ps * P:(ps + 1) * P], in_=tf)

        # khT[e', pk] = Wk^T @ VPT + rank-1 bias rows
        with tc.tile_pool(name="khp", bufs=2, space="PSUM") as khp:
            for pr in range(4):
                ps_t = khp.tile([P, PK], F32, tag="kh")
                for ec in range(4):
                    nc.tensor.matmul(
                        ps_t, lhsT=wk_sb[:, ec, pr * P:(pr + 1) * P],
                        rhs=vpT[:, ec, :], start=(ec == 0), stop=False)
                nc.tensor.matmul(
                    ps_t, lhsT=wkaug_sb[:, pr * P:(pr + 1) * P],
                    rhs=auge_sb, start=False, stop=True)
                nc.vector.tensor_copy(out=khT[:, pr, :], in_=ps_t)

        # vh[pk, dv] = VFT^T @ Wv + rank-1 bias rows (seq-major in pk)
        with tc.tile_pool(name="vhp", bufs=2, space="PSUM") as vhp:
            for ps in range(2):
                ps_t = vhp.tile([P, D], F32, tag="vh")
                for ec in range(4):
                    nc.tensor.matmul(
                        ps_t, lhsT=vfT[:, ec, ps * P:(ps + 1) * P],
                        rhs=wv_sb[:, ec, :], start=(ec == 0), stop=False)
                nc.tensor.matmul(
                    ps_t, lhsT=augf_sb[:, ps * P:(ps + 1) * P],
                    rhs=wvaug_sb, start=False, stop=True)
                nc.vector.tensor_copy(
                    out=vh_sb[:, ps, :, :],
                    in_=ps_t.rearrange("p (h v) -> p h v", h=H))

        # ---- pair AllReduce of the half-sequence partials of khT and vh
        # (the rank-1 bias rows were halved on the host so the pair sum
        # applies them exactly once) ----
        with tc.tile_pool(name="dramb", bufs=2, space="DRAM") as dramb:
            cc_in = dramb.tile([P, 2048], BF16, name="cc_in")
            cc_out = dramb.tile([P, 2048], BF16, name="cc_out")
            nc.gpsimd.dma_start(out=cc_in[:, 0:1024],
                                in_=khT.rearrange("p a k -> p (a k)"))
            nc.gpsimd.dma_start(out=cc_in[:, 1024:2048],
                                in_=vh_sb.rearrange("p c h v -> p (c h v)"))
            nc.gpsimd.collective_compute(
                "AllReduce", OP.add,
                replica_groups=[[0, 1], [2, 3], [4, 5], [6, 7]],
                ins=[cc_in.opt()], outs=[cc_out.opt()])
            nc.gpsimd.dma_start(out=khT.rearrange("p a k -> p (a k)"),
                                in_=cc_out[:, 0:1024])
            nc.gpsimd.dma_start(out=vh_sb.rearrange("p c h v -> p (c h v)"),
                                in_=cc_out[:, 1024:2048])

        # ---- attention: per s-tile, q-projection interleaved with
        #      pair-packed scores / softmax / AV / output projection.
        # One shared 4-slot PSUM ring carries qh, score, and Wo tiles (all
        # [P,512]); nz gets 2 double-bank slots; total exactly 8 banks.
        # Emission is software-pipelined: scores run up to two pairs ahead
        # of the AV matmuls so the serial exp latency of pair j overlaps
        # PE work of pairs j+1/j+2 despite in-order engine queues.
        out_r = out_t.rearrange("(t c p) d -> t p c d", c=4, p=P)
        with (
            tc.tile_pool(name="spool", bufs=4, space="PSUM") as spool,  # 4 banks
            tc.tile_pool(name="nzp", bufs=2, space="PSUM") as nzp,      # 4 banks
            tc.tile_pool(name="qstp", bufs=8) as qstp,
            tc.tile_pool(name="epool", bufs=8) as epool,
            tc.tile_pool(name="rzp", bufs=2) as rzp,
            tc.tile_pool(name="avp", bufs=2) as avp,
            tc.tile_pool(name="ostage", bufs=2) as ostage,
        ):
            # Wo for s-tile st-1 is emitted in the middle of s-tile st so
            # the PE's output-projection block lands inside the next tile's
            # exp-saturated stretch (the scalar engine is the attention
            # pacer and must never idle).
            def emit_wo(wst, wav):
                o_sb = ostage.tile([P, 4, D], F32, tag="ost",
                                   name=f"osb{wst}")
                for sl in range(4):
                    o_t = spool.tile([P, D], F32, tag="s", name=f"ot{wst}_{sl}")
                    for pr in range(4):
                        nc.tensor.matmul(
                            o_t, lhsT=wav[:, pr, sl * P:(sl + 1) * P],
                            rhs=wo_sb[:, pr, :], start=(pr == 0), stop=(pr == 3))
                    nc.vector.tensor_copy(out=o_sb[:, sl, :], in_=o_t)
                nc.sync.dma_start(out=out_r[wst], in_=o_sb)

            # All q-projection chains are hoisted ahead of the s-tile
            # loop: their matmuls fill the PE while the pair-AllReduce of
            # khT/vh is in flight, and their PSUM->SBUF copies (with the
            # bq bias) ride the scalar engine, which is idle until the
            # first exp.
            qsts = []
            for st in range(NT):
                ssl = slice(st * 512, (st + 1) * 512)
                qst = qstp.tile([P, 4, 512], BF16, tag="qst", name=f"qst{st}")
                qsts.append(qst)
                for j in range(4):
                    qt = spool.tile([P, 512], F32, tag="s", name=f"qt{st}_{j}")
                    for dc in range(4):
                        nc.tensor.matmul(
                            qt, lhsT=wq_sb[:, dc, j * P:(j + 1) * P],
                            rhs=qTraw[:, dc, ssl],
                            start=(dc == 0), stop=(dc == 3))
                    nc.scalar.activation(
                        out=qst[:, j, :], in_=qt, func=AF.Identity,
                        bias=bq_sb[:, j:j + 1])

            pend = None
            for st in range(NT):
                qst = qsts[st]
                av_sb = avp.tile([P, 4, 512], BF16, tag="av")

                es = {}

                def emit_scores(j):
                    pe = []
                    for ps in range(2):
                        psl = slice(ps * P, (ps + 1) * P)
                        scA = spool.tile([P, 512], F32, tag="s",
                                         name=f"scA{st}_{j}_{ps}")
                        scB = spool.tile([P, 512], F32, tag="s",
                                         name=f"scB{st}_{j}_{ps}")
                        nc.tensor.matmul(
                            scA, lhsT=khT[0:64, j, psl],
                            rhs=qst[0:64, j, :], start=True, stop=True,
                            tile_position=(0, 0))
                        nc.tensor.matmul(
                            scB, lhsT=khT[64:P, j, psl],
                            rhs=qst[64:P, j, :], start=True, stop=True,
                            tile_position=(64, 0))
                        eA = epool.tile([P, 512], BF16, tag="e",
                                        name=f"eA{st}_{j}_{ps}")
                        eB = epool.tile([P, 512], BF16, tag="e",
                                        name=f"eB{st}_{j}_{ps}")
                        nc.scalar.activation(out=eA, in_=scA, func=AF.Exp)
                        nc.scalar.activation(out=eB, in_=scB, func=AF.Exp)
                        pe.append((eA, eB))
                    es[j] = pe

                def emit_av(j):
                    # AV + denominator: pair stacked into one PSUM tile;
                    # nz[:,0,:] = numerator, nz[:,1,:] = Z (dup x64)
                    nz = nzp.tile([P, 2, 512], F32, tag="nz", name=f"nz{st}_{j}")
                    for c in range(2):
                        fl, ll = (c == 0), (c == 1)
                        eA, eB = es[j][c]
                        nc.tensor.matmul(
                            nz[0:64, 0, :], lhsT=vh_sb[:, c, 2 * j, :],
                            rhs=eA, start=fl, stop=ll, tile_position=(0, 0))
                        nc.tensor.matmul(
                            nz[64:P, 0, :], lhsT=vh_sb[:, c, 2 * j + 1, :],
                            rhs=eB, start=fl, stop=ll, tile_position=(0, 64))
                        nc.tensor.matmul(
                            nz[0:64, 1, :], lhsT=ones64[:, :],
                            rhs=eA, start=fl, stop=ll, tile_position=(0, 0))
                        nc.tensor.matmul(
                            nz[64:P, 1, :], lhsT=ones64[:, :],
                            rhs=eB, start=fl, stop=ll, tile_position=(0, 64))
                    rz = rzp.tile([P, 512], F32, tag="rz", name=f"rz{st}_{j}")
                    nc.vector.reciprocal_approx_fast(out=rz, in_=nz[:, 1, :])
                    nc.vector.tensor_tensor(
                        out=av_sb[:, j, :], in0=nz[:, 0, :], in1=rz,
                        op=OP.mult)

                emit_scores(0)
                emit_scores(1)
                if pend is not None:
                    emit_wo(*pend)
                    pend = None
                emit_scores(2)
                emit_av(0)
                emit_scores(3)
                emit_av(1)
                emit_av(2)
                emit_av(3)
                pend = (st, av_sb)
            emit_wo(*pend)

    nc.finalize()
    return nc


def _prep_inputs(inputs):
    bf = ml_dtypes.bfloat16
    f32 = np.float32
    q = np.ascontiguousarray(inputs["query"])
    v = np.ascontiguousarray(inputs["value"])
    We, Wf = np.asarray(inputs["We"]), np.asarray(inputs["Wf"])
    scale = np.float32(DK ** -0.5)
    ones = np.ones(D, f32)
    sWe = We.astype(f32).sum(0)
    sWf = Wf.astype(f32).sum(0)
    # the rank-1 bias rows are applied on BOTH cores of a batch pair and
    # then pair-AllReduced, so they carry a factor 1/2 here
    shared = {
        "wq": (np.asarray(inputs["Wq"]) * scale).astype(bf),
        "wk": np.asarray(inputs["Wk"]).astype(bf),
        "wv": np.asarray(inputs["Wv"]).astype(bf),
        "wo": np.asarray(inputs["Wo"]).astype(bf),
        "wkaug": np.stack([np.asarray(inputs["bk"], f32), ones]).astype(bf),
        "auge": (0.5 * np.stack([sWe, np.asarray(inputs["be"], f32)])).astype(bf),
        "wvaug": np.stack([np.asarray(inputs["bv"], f32), ones]).astype(bf),
        "augf": (0.5 * np.stack([sWf, np.asarray(inputs["bf"], f32)])).astype(bf),
        "bq": (np.asarray(inputs["bq"]) * scale).astype(f32),
    }
    we_h = [np.ascontiguousarray(We[h * SH:(h + 1) * SH]).astype(bf)
            for h in range(2)]
    wf_h = [np.ascontiguousarray(Wf[h * SH:(h + 1) * SH]).astype(bf)
            for h in range(2)]
    in_maps = []
    for c in range(NCORES):
        b, half = c // 2, c % 2
        m = dict(shared)
        m["q"] = np.ascontiguousarray(q[b, half * SH:(half + 1) * SH, :]).astype(bf)
        m["v"] = np.ascontiguousarray(v[b, half * SH:(half + 1) * SH, :]).astype(bf)
        m["we"] = we_h[half]
        m["wf"] = wf_h[half]
        in_maps.append(m)
    return in_maps


def kernel(**inputs):
    if "nc" not in _CACHE:
        _CACHE["nc"] = _build_kernel()
    nc = _CACHE["nc"]
    in_maps = _prep_inputs(inputs)
    res = bass_utils.run_bass_kernel_spmd(nc, in_maps, core_ids=list(range(NCORES)))
    bo = np.asarray(inputs["bo"], np.float32)
    out = np.empty((B, S, D), np.float32)
    for c in range(NCORES):
        b, half = c // 2, c % 2
        out[b, half * SH:(half + 1) * SH, :] = res.results[c]["out"]
    out += bo
    return out
